# revision 1
# baseline (speedup 1.0000x reference)
"""GeneSAGE (2-layer GraphSAGE + skip + LayerNorm + ELU) on 8 Trainium2 cores.

Strategy: edge-parallel by *destination range*. Core c owns nodes
[CP*c, CP*(c+1)) with CP=6272 (=49*128). Edges are bucketed host-side by
(dst-core, src-half, dst-window) and padded to 128-edge chunks with a chunk
structure common to all 8 cores (SPMD: one program). On device, per chunk:
one-hot(dst) built on DVE, segment-sum done as one-hot matmuls accumulating
in PSUM per 128-node window. Features are gathered from HBM with
dma_gather (256B rows, int16 indices over two half tables). Conv2 gathers
from an on-device-built table pb2 whose rows replicate p=h@W2l 32x so rows
are 256B. The only collective is an AllGather of p^T (2 x 6272 per core).
"""

import numpy as np

import concourse.mybir as mybir
from concourse import bacc, bass, tile
from concourse.bass_utils import run_bass_kernel_spmd

F32 = mybir.dt.float32
I16 = mybir.dt.int16

N_CORES = 8
D = 64          # input feature dim
HID = 256
OUT = 2
LN_EPS = 1e-5
BATCH_CHUNKS = 32   # chunks per dma_gather call
STAGE_CHUNKS = 0    # set per-plan: pb2 staging chunks per DMA


def make_plan(edge_index: np.ndarray, n_nodes: int):
    """Host-side edge bucketing. Returns plan dict with per-core arrays and
    the (common) chunk schedule."""
    # padded per-core node count: multiple of 128, 8 cores cover all nodes
    cp = int(np.ceil(n_nodes / (N_CORES * 128))) * 128
    nw = cp // 128
    npad = N_CORES * cp
    half = npad // 2
    assert half <= 32768, "int16 gather index limit"

    src = edge_index[0].astype(np.int64)
    dst = edge_index[1].astype(np.int64)
    E = src.shape[0]

    core = dst // cp
    stream = (src >= half).astype(np.int64)
    win = (dst % cp) // 128
    ngrp_per_core = 2 * nw
    key = (core * 2 + stream) * nw + win
    order = np.argsort(key, kind="stable")
    counts = np.bincount(key, minlength=N_CORES * ngrp_per_core).reshape(
        N_CORES, 2, nw
    )
    # common chunk count per (stream, window): max over cores
    nchunks = -(-counts.max(axis=0) // 128)  # [2, nw] ceil-div
    # chunk offsets, stream-major
    off = np.zeros((2, nw), np.int64)
    running = 0
    for s in range(2):
        for w in range(nw):
            off[s, w] = running
            running += nchunks[s, w]
    c_total = int(running)
    c_lo = int(nchunks[0].sum())
    e_slots = c_total * 128

    # per-edge slot position
    sk = key[order]
    grp_start = np.searchsorted(sk, np.arange(N_CORES * ngrp_per_core))
    rank = np.arange(E) - grp_start[sk]
    s_of = (sk // nw) % 2
    w_of = sk % nw
    c_of = sk // ngrp_per_core
    slot = off[s_of, w_of] * 128 + rank

    gidx = np.zeros((N_CORES, e_slots), np.int16)
    dstf = np.full((N_CORES, e_slots), -1.0, np.float32)
    gidx[c_of, slot] = (src[order] - s_of * half).astype(np.int16)
    dstf[c_of, slot] = (dst[order] % cp - w_of * 128).astype(np.float32)

    # device layouts
    # gather idx tile [128, e_slots//16]: tile[p, j] = gidx[16*j + p%16]
    a = gidx.reshape(N_CORES, e_slots // 16, 16).transpose(0, 2, 1)  # [c,16,J]
    gidx_tile = np.tile(a, (1, 8, 1)).copy()  # [c, 128, J]
    dstf_tile = (
        dstf.reshape(N_CORES, c_total, 128).transpose(0, 2, 1).copy()
    )  # [c, 128, C]

    # per (stream, window) first/last chunk ids (global chunk index), or None
    sched = []  # list over streams of list of (window, first_chunk, last_chunk)
    for s in range(2):
        rows = []
        for w in range(nw):
            n = int(nchunks[s, w])
            if n == 0:
                continue
            first = int(off[s, w])
            rows.append((w, first, first + n - 1))
        sched.append(rows)

    chunk_window = np.zeros(c_total, np.int64)
    for s in range(2):
        for w, f, l in sched[s]:
            chunk_window[f : l + 1] = w

    return dict(
        cp=cp, nw=nw, npad=npad, half=half,
        c_total=c_total, c_lo=c_lo,
        sched=sched, chunk_window=chunk_window,
        gidx_tile=gidx_tile, dstf_tile=dstf_tile,
        nchunks=nchunks,
    )


def build_program(plan):
    cp, nw, half = plan["cp"], plan["nw"], plan["half"]
    c_total, c_lo = plan["c_total"], plan["c_lo"]
    sched, chunk_window = plan["sched"], plan["chunk_window"]
    J = c_total * 8

    nc = bacc.Bacc("TRN2", target_bir_lowering=False, debug=False,
                   num_devices=N_CORES)

    def inp(name, shape, dt=F32):
        return nc.dram_tensor(name, shape, dt, kind="ExternalInput").ap()

    x_lo = inp("x_lo", [half, D])
    x_hi = inp("x_hi", [half, D])
    x_loc = inp("x_loc", [cp, D])
    gidx_d = inp("gidx", [128, J], I16)
    dstf_d = inp("dstf", [128, c_total])
    iota_d = inp("iota", [128, 128])
    ident_d = inp("ident", [128, 128])
    wcb_d = inp("wcb", [D + 1, HID])       # [Wc; bc]
    w1l_d = inp("w1l", [D, HID])
    w2lr_d = inp("w2lr", [128, 2 * 2 * OUT])  # halves of [W2l|W2r] packed
    gamma_d = inp("gamma_bc", [128, HID])
    beta_d = inp("beta_bc", [128, HID])
    b2_d = inp("b2_bc", [128, OUT])
    i2_d = inp("i2", [2, D])               # interleave pattern
    out_d = nc.dram_tensor("out", [cp, OUT], F32, kind="ExternalOutput").ap()

    with tile.TileContext(nc) as tc:
        with (
            tc.tile_pool(name="res", bufs=1) as res,
            tc.tile_pool(name="dram", bufs=1, space="DRAM") as dram,
        ):
            # ---- resident tiles / constants
            gidx_sb = res.tile([128, J], I16)
            nc.sync.dma_start(out=gidx_sb[:], in_=gidx_d[:])
            dstf_sb = res.tile([128, c_total], F32)
            nc.sync.dma_start(out=dstf_sb[:], in_=dstf_d[:])
            iota_sb = res.tile([128, 128], F32)
            nc.sync.dma_start(out=iota_sb[:], in_=iota_d[:])
            ident_sb = res.tile([128, 128], F32)
            nc.sync.dma_start(out=ident_sb[:], in_=ident_d[:])
            wcb_sb = res.tile([D + 1, HID], F32)
            nc.sync.dma_start(out=wcb_sb[:], in_=wcb_d[:])
            w1l_sb = res.tile([D, HID], F32)
            nc.sync.dma_start(out=w1l_sb[:], in_=w1l_d[:])
            w2lr_sb = res.tile([128, 2 * 2 * OUT], F32)
            nc.sync.dma_start(out=w2lr_sb[:], in_=w2lr_d[:])
            gamma_sb = res.tile([128, HID], F32)
            nc.sync.dma_start(out=gamma_sb[:], in_=gamma_d[:])
            beta_sb = res.tile([128, HID], F32)
            nc.sync.dma_start(out=beta_sb[:], in_=beta_d[:])
            b2_sb = res.tile([128, OUT], F32)
            nc.sync.dma_start(out=b2_sb[:], in_=b2_d[:])
            i2_sb = res.tile([2, D], F32)
            nc.sync.dma_start(out=i2_sb[:], in_=i2_d[:])
            xloc_sb = res.tile([128, nw, D], F32)
            nc.sync.dma_start(
                out=xloc_sb[:],
                in_=x_loc.rearrange("(w p) d -> p w d", p=128),
            )
            ones_sb = res.tile([128, 1], F32)
            nc.vector.memset(ones_sb[:], 1.0)

            aggcnt = res.tile([128, nw, D + 1], F32)
            nc.vector.memset(aggcnt[:], 0.0)
            agg2 = res.tile([128, nw, OUT], F32)
            nc.vector.memset(agg2[:], 0.0)
            rc_sb = res.tile([128, nw, 1], F32)
            h_sb = res.tile([128, nw, HID], F32)
            pr_sb = res.tile([128, nw, 2 * OUT], F32)
            out_sb = res.tile([128, nw, OUT], F32)

            pt_dram = dram.tile([2, cp], F32)
            ptall_dram = dram.tile([2 * N_CORES, cp], F32)
            pb2_lo = dram.tile([half, D], F32)
            pb2_hi = dram.tile([half, D], F32)

            # ---- generic aggregation pass over the edge stream
            def aggregation(tables, acc_tile, width, with_cnt):
                """tables: (lo_ap, hi_ap); acc_tile[:, w, :] accumulated into.
                width: feature width gathered (cols 0:width of 64 used).
                with_cnt: also matmul ones into acc col `width`."""
                with (
                    tc.tile_pool(name="gpool", bufs=3) as gpool,
                    tc.tile_pool(name="opool", bufs=4) as opool,
                    tc.tile_pool(name="pwpool", bufs=2, space="PSUM") as pwp,
                ):
                    for s in range(2):
                        table = tables[s]
                        rows = sched[s]
                        if not rows:
                            continue
                        c0 = rows[0][1]
                        c1 = rows[-1][2] + 1
                        # gather batches
                        gbufs = {}
                        for b0 in range(c0, c1, BATCH_CHUNKS):
                            b1 = min(b0 + BATCH_CHUNKS, c1)
                            g = gpool.tile([128, BATCH_CHUNKS, D], F32,
                                           tag="gbuf")
                            n_idx = (b1 - b0) * 128
                            nc.gpsimd.dma_gather(
                                out_ap=g[:, 0 : b1 - b0, :],
                                in_ap=table,
                                idxs_ap=gidx_sb[:, b0 * 8 : b1 * 8],
                                num_idxs=n_idx,
                                num_idxs_reg=n_idx,
                                elem_size=D,
                                single_packet=False,
                            )
                            gbufs[b0] = g
                        # per-window accumulation
                        for w, first, last in rows:
                            pw = pwp.tile([128, D], F32, tag="pw")
                            pwc = None
                            if with_cnt:
                                pwc = pwp.tile([128, 1], F32, tag="pwc",
                                               name="pwc")
                            for g in range(first, last + 1):
                                b0 = c0 + ((g - c0) // BATCH_CHUNKS) * BATCH_CHUNKS
                                gb = gbufs[b0]
                                o = opool.tile([128, 128], F32, tag="O")
                                nc.vector.tensor_scalar(
                                    out=o[:],
                                    in0=iota_sb[:],
                                    scalar1=dstf_sb[:, g : g + 1],
                                    scalar2=None,
                                    op0=mybir.AluOpType.is_equal,
                                )
                                st = g == first
                                sp = g == last
                                nc.tensor.matmul(
                                    pw[:, 0:width], o[:],
                                    gb[:, g - b0, 0:width],
                                    start=st, stop=sp,
                                )
                                if with_cnt:
                                    nc.tensor.matmul(
                                        pwc[:], o[:],
                                        ones_sb[:], start=st, stop=sp,
                                    )
                            nc.vector.tensor_tensor(
                                out=acc_tile[:, w, 0:width],
                                in0=acc_tile[:, w, 0:width],
                                in1=pw[:, 0:width],
                                op=mybir.AluOpType.add,
                            )
                            if with_cnt:
                                nc.vector.tensor_tensor(
                                    out=acc_tile[:, w, width : width + 1],
                                    in0=acc_tile[:, w, width : width + 1],
                                    in1=pwc[:],
                                    op=mybir.AluOpType.add,
                                )

            # ================= conv1 aggregation =================
            aggregation((x_lo, x_hi), aggcnt, D, True)

            # ================= dense phase =================
            with (
                tc.tile_pool(name="dwork", bufs=3) as dwork,
                tc.tile_pool(name="dsmall", bufs=4) as dsmall,
                tc.tile_pool(name="dpsum", bufs=2, space="PSUM") as dpsum,
                tc.tile_pool(name="dpsum2", bufs=1, space="PSUM") as dpsum2,
                tc.tile_pool(name="ptpool", bufs=1) as ptpool,
            ):
                pt_sb = ptpool.tile([2, cp], F32)
                for n in range(nw):
                    # reciprocal of count (and save for conv2)
                    cmax = dsmall.tile([128, 1], F32, tag="cmax")
                    nc.vector.tensor_scalar(
                        out=cmax[:], in0=aggcnt[:, n, D : D + 1],
                        scalar1=1.0, scalar2=None, op0=mybir.AluOpType.max,
                    )
                    nc.vector.reciprocal(rc_sb[:, n, :], cmax[:])
                    mean_n = dwork.tile([128, D], F32, tag="mean")
                    nc.vector.tensor_scalar(
                        out=mean_n[:], in0=aggcnt[:, n, 0:D],
                        scalar1=rc_sb[:, n, :], scalar2=None,
                        op0=mybir.AluOpType.mult,
                    )
                    # transposes: x_loc chunk and mean chunk -> feature major
                    tp = dpsum.tile([D, 128], F32, tag="tp")
                    nc.tensor.transpose(tp[:], xloc_sb[:, n, :], ident_sb[:])
                    xto = dwork.tile([D + 1, 128], F32, tag="xto")
                    nc.scalar.activation(
                        xto[0:D, :], tp[:], mybir.ActivationFunctionType.Copy)
                    nc.vector.memset(xto[D : D + 1, :], 1.0)
                    tp2 = dpsum.tile([D, 128], F32, tag="tp")
                    nc.tensor.transpose(tp2[:], mean_n[:], ident_sb[:])
                    meant = dwork.tile([D, 128], F32, tag="meant")
                    nc.scalar.activation(
                        meant[:], tp2[:], mybir.ActivationFunctionType.Copy)

                    # x1 = x@Wc + bc + mean@W1l   [128, HID]
                    x1p = dpsum2.tile([128, HID], F32, tag="x1")
                    nc.tensor.matmul(x1p[:], xto[:], wcb_sb[:],
                                     start=True, stop=False)
                    nc.tensor.matmul(x1p[:], meant[:], w1l_sb[:],
                                     start=False, stop=True)

                    # LayerNorm + ELU (node-major, free-dim reductions)
                    mu = dsmall.tile([128, 1], F32, tag="mu")
                    nc.vector.reduce_sum(out=mu[:], in_=x1p[:], axis=mybir.AxisListType.X)
                    nc.vector.tensor_scalar(
                        out=mu[:], in0=mu[:], scalar1=1.0 / HID,
                        scalar2=None, op0=mybir.AluOpType.mult)
                    xc = dwork.tile([128, HID], F32, tag="xc")
                    nc.vector.tensor_scalar(
                        out=xc[:], in0=x1p[:], scalar1=mu[:], scalar2=None,
                        op0=mybir.AluOpType.subtract)
                    sq = dwork.tile([128, HID], F32, tag="sq")
                    var = dsmall.tile([128, 1], F32, tag="var")
                    nc.scalar.activation(
                        sq[:], xc[:], mybir.ActivationFunctionType.Square,
                        accum_out=var[:])
                    rstd = dsmall.tile([128, 1], F32, tag="rstd")
                    nc.vector.tensor_scalar(
                        out=rstd[:], in0=var[:], scalar1=1.0 / HID,
                        scalar2=LN_EPS, op0=mybir.AluOpType.mult,
                        op1=mybir.AluOpType.add)
                    nc.scalar.activation(
                        rstd[:], rstd[:], mybir.ActivationFunctionType.Sqrt)
                    nc.vector.reciprocal(rstd[:], rstd[:])
                    y = dwork.tile([128, HID], F32, tag="y")
                    nc.vector.tensor_scalar(
                        out=y[:], in0=xc[:], scalar1=rstd[:], scalar2=None,
                        op0=mybir.AluOpType.mult)
                    nc.vector.tensor_tensor(
                        out=y[:], in0=y[:], in1=gamma_sb[:],
                        op=mybir.AluOpType.mult)
                    nc.vector.tensor_tensor(
                        out=y[:], in0=y[:], in1=beta_sb[:],
                        op=mybir.AluOpType.add)
                    # ELU: h = max(y,0)-1 + exp(min(y,0))
                    m0 = dwork.tile([128, HID], F32, tag="m0")
                    nc.vector.tensor_scalar(
                        out=m0[:], in0=y[:], scalar1=0.0, scalar2=None,
                        op0=mybir.AluOpType.min)
                    ex = dwork.tile([128, HID], F32, tag="ex")
                    nc.scalar.activation(
                        ex[:], m0[:], mybir.ActivationFunctionType.Exp)
                    rm1 = dwork.tile([128, HID], F32, tag="rm1")
                    nc.vector.tensor_scalar(
                        out=rm1[:], in0=y[:], scalar1=0.0, scalar2=-1.0,
                        op0=mybir.AluOpType.max, op1=mybir.AluOpType.add)
                    nc.vector.tensor_tensor(
                        out=h_sb[:, n, :], in0=rm1[:], in1=ex[:],
                        op=mybir.AluOpType.add)

                    # p | r = h @ [W2l | W2r]
                    prp = dpsum2.tile([128, 2 * OUT], F32, tag="pr")
                    for hh in range(2):
                        tph = dpsum.tile([128, 128], F32, tag="tph")
                        nc.tensor.transpose(
                            tph[:], h_sb[:, n, 128 * hh : 128 * (hh + 1)],
                            ident_sb[:])
                        hts = dwork.tile([128, 128], F32, tag="hts")
                        nc.scalar.activation(
                            hts[:], tph[:],
                            mybir.ActivationFunctionType.Copy)
                        nc.tensor.matmul(
                            prp[:], hts[:],
                            w2lr_sb[:, 4 * hh : 4 * (hh + 1)],
                            start=(hh == 0), stop=(hh == 1))
                    nc.scalar.activation(
                        pr_sb[:, n, :], prp[:],
                        mybir.ActivationFunctionType.Copy)
                    # p^T into [2, cp]
                    ptp = dpsum2.tile([OUT, 128], F32, tag="ptp")
                    nc.tensor.transpose(
                        ptp[:], pr_sb[:, n, 0:OUT], ident_sb[:])
                    nc.scalar.activation(
                        pt_sb[:, 128 * n : 128 * (n + 1)], ptp[:],
                        mybir.ActivationFunctionType.Copy)

                # ================= p all-gather =================
                nc.sync.dma_start(out=pt_dram[:], in_=pt_sb[:])
            nc.gpsimd.collective_compute(
                "AllGather",
                mybir.AluOpType.bypass,
                replica_groups=[list(range(N_CORES))],
                ins=[pt_dram.opt()],
                outs=[ptall_dram.opt()],
            )

            # ================= build pb2 (replicated p table) =================
            n_glob = N_CORES * nw  # global 128-node chunks
            stage_n = 14 if n_glob % 14 == 0 and (n_glob // 2) % 14 == 0 else 1
            half_rows_chunks = half // 128
            with (
                tc.tile_pool(name="bstage", bufs=2) as bstage,
                tc.tile_pool(name="bpt", bufs=2) as bpt,
                tc.tile_pool(name="bpsum", bufs=2, space="PSUM") as bpsum,
            ):
                stage = None
                ptb = None
                for j in range(n_glob):
                    c = j // nw
                    jw = j % nw
                    if jw == 0:
                        ptb = bpt.tile([2, cp], F32, tag="ptb", name="ptb")
                        nc.sync.dma_start(
                            out=ptb[:], in_=ptall_dram[2 * c : 2 * c + 2, :])
                    pp = bpsum.tile([128, D], F32, tag="pb2p")
                    nc.tensor.matmul(
                        pp[:],
                        ptb[:, 128 * jw : 128 * (jw + 1)],
                        i2_sb[:], start=True, stop=True)
                    if j % stage_n == 0:
                        stage = bstage.tile([128, stage_n, D], F32,
                                            tag="stage")
                    nc.scalar.activation(
                        stage[:, j % stage_n, :], pp[:],
                        mybir.ActivationFunctionType.Copy)
                    if j % stage_n == stage_n - 1:
                        j0 = j - stage_n + 1
                        r0 = j0 * 128  # global row
                        if r0 < half:
                            dst = pb2_lo[r0 : r0 + stage_n * 128, :]
                        else:
                            dst = pb2_hi[r0 - half : r0 - half
                                         + stage_n * 128, :]
                        nc.sync.dma_start(
                            out=dst.rearrange("(s p) d -> p s d", p=128),
                            in_=stage[:])

            # ================= conv2 aggregation =================
            aggregation((pb2_lo, pb2_hi), agg2, OUT, False)

            # ================= output =================
            with tc.tile_pool(name="fsmall", bufs=4) as fsmall:
                for n in range(nw):
                    t = fsmall.tile([128, OUT], F32, tag="fo")
                    nc.vector.tensor_scalar(
                        out=t[:], in0=agg2[:, n, :], scalar1=rc_sb[:, n, :],
                        scalar2=None, op0=mybir.AluOpType.mult)
                    nc.vector.tensor_tensor(
                        out=t[:], in0=t[:], in1=pr_sb[:, n, OUT : 2 * OUT],
                        op=mybir.AluOpType.add)
                    nc.vector.tensor_tensor(
                        out=out_sb[:, n, :], in0=t[:], in1=b2_sb[:],
                        op=mybir.AluOpType.add)
            nc.sync.dma_start(
                out=out_d.rearrange("(w p) c -> p w c", p=128),
                in_=out_sb[:])

    nc.compile()
    return nc


def make_inputs(plan, x, W1l, W1r, b1, Wskip, bskip, gamma, beta, W2l, W2r,
                b2, n_nodes):
    cp, half, npad = plan["cp"], plan["half"], plan["npad"]
    xp = np.zeros((npad, D), np.float32)
    xp[:n_nodes] = np.asarray(x, np.float32)
    wc = np.asarray(W1r, np.float32) + np.asarray(Wskip, np.float32)
    bc = np.asarray(b1, np.float32) + np.asarray(bskip, np.float32)
    wcb = np.concatenate([wc, bc[None, :]], axis=0)
    w2lr_full = np.concatenate(
        [np.asarray(W2l, np.float32), np.asarray(W2r, np.float32)], axis=1
    )  # [HID, 4]
    w2lr = (
        w2lr_full.reshape(2, 128, 2 * OUT).transpose(1, 0, 2)
        .reshape(128, 2 * 2 * OUT).copy()
    )
    iota = np.tile(np.arange(128, dtype=np.float32)[None, :], (128, 1))
    ident = np.eye(128, dtype=np.float32)
    i2 = np.zeros((2, D), np.float32)
    i2[0, 0::2] = 1.0
    i2[1, 1::2] = 1.0
    gamma_bc = np.tile(np.asarray(gamma, np.float32)[None, :], (128, 1))
    beta_bc = np.tile(np.asarray(beta, np.float32)[None, :], (128, 1))
    b2_bc = np.tile(np.asarray(b2, np.float32)[None, :], (128, 1))

    common = dict(
        x_lo=xp[:half].copy(), x_hi=xp[half:].copy(),
        iota=iota, ident=ident,
        wcb=wcb, w1l=np.asarray(W1l, np.float32), w2lr=w2lr,
        gamma_bc=gamma_bc, beta_bc=beta_bc, b2_bc=b2_bc, i2=i2,
    )
    in_maps = []
    for c in range(N_CORES):
        m = dict(common)
        m["x_loc"] = xp[cp * c : cp * (c + 1)].copy()
        m["gidx"] = plan["gidx_tile"][c]
        m["dstf"] = plan["dstf_tile"][c]
        in_maps.append(m)
    return in_maps


_CACHE = {}


def _get_compiled(edge_index, n_nodes):
    key = (edge_index.tobytes()[:512], edge_index.shape, n_nodes)
    if key not in _CACHE:
        plan = make_plan(edge_index, n_nodes)
        nc = build_program(plan)
        _CACHE[key] = (plan, nc)
    return _CACHE[key]


def run(inputs, trace=False):
    x = np.asarray(inputs["x"], np.float32)
    edge_index = np.asarray(inputs["edge_index"], np.int32)
    n_nodes = x.shape[0]
    plan, nc = _get_compiled(edge_index, n_nodes)
    in_maps = make_inputs(
        plan, x, inputs["W1l"], inputs["W1r"], inputs["b1"], inputs["Wskip"],
        inputs["bskip"], inputs["gamma"], inputs["beta"], inputs["W2l"],
        inputs["W2r"], inputs["b2"], n_nodes)
    res = run_bass_kernel_spmd(
        nc, in_maps, list(range(N_CORES)), trace=trace)
    cp = plan["cp"]
    out = np.empty((n_nodes, OUT), np.float32)
    for c in range(N_CORES):
        lo = cp * c
        hi = min(cp * (c + 1), n_nodes)
        out[lo:hi] = res.results[c]["out"][0 : hi - lo]
    return out, res


def kernel(**inputs) -> np.ndarray:
    out, _ = run(inputs)
    return out



# revision 9
# speedup vs baseline: 1.1545x; 1.1545x over previous
"""GeneSAGE (2-layer GraphSAGE + skip + LayerNorm + ELU) on 8 Trainium2 cores.

V2 design. Edge-parallel by destination range: core c owns dst nodes
[cp*c, cp*(c+1)), cp=6272. Edges bucketed host-side by (dst-core, src-half,
dst-window) into 128-edge chunks with an SPMD-common chunk schedule.

Key speed choices (measured on HW):
- Gathered x is stored hi/lo bf16 split ([bf16(x) | bf16(x-bf16(x))] 256B
  rows) so the aggregation matmul runs in bf16 (1 instr, ~130ns) while
  keeping fp32-class accuracy (products exact vs 0/1 one-hot; PSUM fp32).
- One-hot built on DVE via is_equal (bf16 out, fp32 iota/scalar) ~160ns.
- Aggregation matmul operand-swapped: lhsT = gathered tile, rhs = one-hot
  -> PSUM holds aggT (feature-major), killing the dense-phase transposes.
- Edge counts (degrees) precomputed on host; no count matmuls.
- dma_gather calls round-robin over 2 SWDGE queues: descriptor generation
  of the next call overlaps the wait of the current (7.9 -> ~4.7 ns/idx).
- conv2 gathers a [p_hi | p_lo]-replicated 256B-row table (built on device
  from an AllGather of p^T hi/lo parts); one bf16 matmul per chunk.
"""

import numpy as np

import concourse.mybir as mybir
from concourse import bacc, bass, tile
from concourse.bass_utils import run_bass_kernel_spmd

F32 = mybir.dt.float32
BF16 = mybir.dt.bfloat16
I16 = mybir.dt.int16

N_CORES = 8
D = 64          # input feature dim
HID = 256
OUT = 2
LN_EPS = 1e-5
BATCH_CHUNKS = 32   # chunks per dma_gather call
NSWQ = 2            # SWDGE queues (2 parallel descriptor-gen cores)


def make_plan(edge_index: np.ndarray, n_nodes: int):
    """Host-side edge bucketing + degree counts. SPMD-common schedule."""
    cp = int(np.ceil(n_nodes / (N_CORES * 128))) * 128
    nw = cp // 128
    npad = N_CORES * cp
    half = npad // 2
    assert half <= 32768, "int16 gather index limit"

    src = edge_index[0].astype(np.int64)
    dst = edge_index[1].astype(np.int64)
    E = src.shape[0]

    core = dst // cp
    stream = (src >= half).astype(np.int64)
    win = (dst % cp) // 128
    ngrp_per_core = 2 * nw
    key = (core * 2 + stream) * nw + win
    order = np.argsort(key, kind="stable")
    counts = np.bincount(key, minlength=N_CORES * ngrp_per_core).reshape(
        N_CORES, 2, nw
    )
    nchunks = -(-counts.max(axis=0) // 128)  # [2, nw]
    off = np.zeros((2, nw), np.int64)
    running = 0
    for s in range(2):
        for w in range(nw):
            off[s, w] = running
            running += nchunks[s, w]
    c_total = int(running)
    e_slots = c_total * 128

    sk = key[order]
    grp_start = np.searchsorted(sk, np.arange(N_CORES * ngrp_per_core))
    rank = np.arange(E) - grp_start[sk]
    s_of = (sk // nw) % 2
    w_of = sk % nw
    c_of = sk // ngrp_per_core
    slot = off[s_of, w_of] * 128 + rank

    gidx = np.zeros((N_CORES, e_slots), np.int16)
    dstf = np.full((N_CORES, e_slots), -1.0, np.float32)
    gidx[c_of, slot] = (src[order] - s_of * half).astype(np.int16)
    dstf[c_of, slot] = (dst[order] % cp - w_of * 128).astype(np.float32)

    # gather idx tile [128, e_slots//16*8]: tile[p, j] = gidx[16*j + p%16]
    a = gidx.reshape(N_CORES, e_slots // 16, 16).transpose(0, 2, 1)
    gidx_tile = np.tile(a, (1, 8, 1)).copy()  # [c, 128, J]
    dstf_tile = (
        dstf.reshape(N_CORES, c_total, 128).transpose(0, 2, 1).copy()
    )  # [c, 128, C]

    sched = []
    for s in range(2):
        rows = []
        for w in range(nw):
            n = int(nchunks[s, w])
            if n == 0:
                continue
            first = int(off[s, w])
            rows.append((w, first, first + n - 1))
        sched.append(rows)

    # degree counts per node (for mean); reciprocal, per-core window tiles
    cnt = np.bincount(dst, minlength=npad).astype(np.float32)
    rc = 1.0 / np.maximum(cnt, 1.0)
    # rc_tile[c][p, w] = rc[cp*c + 128*w + p]
    rc_tile = rc.reshape(N_CORES, nw, 128).transpose(0, 2, 1).copy()
    # rc broadcast tile for aggT scaling: [128, cp] where [p, 128*w+j] =
    # rc[cp*c + 128*w + j]  (same value down all partitions)
    rc_bc = np.broadcast_to(
        rc.reshape(N_CORES, 1, cp), (N_CORES, 128, cp)
    ).copy()

    return dict(
        cp=cp, nw=nw, npad=npad, half=half,
        c_total=c_total,
        sched=sched,
        gidx_tile=gidx_tile, dstf_tile=dstf_tile,
        rc_tile=rc_tile, rc_bc=rc_bc,
    )


def build_program(plan):
    cp, nw, half = plan["cp"], plan["nw"], plan["half"]
    c_total = plan["c_total"]
    sched = plan["sched"]
    J = c_total * 8
    half_w = half // 128  # 128-row blocks per half table

    nc = bacc.Bacc("TRN2", target_bir_lowering=False, debug=False,
                   num_devices=N_CORES, num_swdge_queues=NSWQ)

    def inp(name, shape, dt=F32):
        return nc.dram_tensor(name, shape, dt, kind="ExternalInput").ap()

    xhl_lo = inp("xhl_lo", [half, 128], BF16)   # [x_hi | x_lo] rows
    xhl_hi = inp("xhl_hi", [half, 128], BF16)
    gidx_d = inp("gidx", [128, J], I16)
    dstf_d = inp("dstf", [128, c_total])
    iota_d = inp("iota", [128, 128])
    xt1_d = inp("xt1", [D + 1, cp])             # x_loc^T with ones row
    wcb_d = inp("wcb", [D + 1, HID])            # [Wc; bc] fp32
    w1l_d = inp("w1l", [D, HID])
    w2lr_d = inp("w2lr", [128, 2 * 2 * OUT])    # packed halves [W2l|W2r]
    ident_d = inp("ident", [128, 128])
    gamma_d = inp("gamma_bc", [128, HID])
    beta_d = inp("beta_bc", [128, HID])
    b2_d = inp("b2_bc", [128, OUT])
    rc_d = inp("rc", [128, nw])
    rcbc_d = inp("rc_bc", [128, cp])
    pat4_d = inp("pat4", [4, 128], BF16)              # p4 tiling pattern
    out_d = nc.dram_tensor("out", [cp, OUT], F32, kind="ExternalOutput").ap()

    with tile.TileContext(nc) as tc:
        with (
            tc.tile_pool(name="res", bufs=1) as res,
            tc.tile_pool(name="dram", bufs=1, space="DRAM") as dram,
        ):
            # ---- resident tiles
            gidx_sb = res.tile([128, J], I16)
            nc.sync.dma_start(out=gidx_sb[:], in_=gidx_d[:])
            dstf_sb = res.tile([128, c_total], F32)
            nc.sync.dma_start(out=dstf_sb[:], in_=dstf_d[:])
            iota_sb = res.tile([128, 128], F32)
            nc.sync.dma_start(out=iota_sb[:], in_=iota_d[:])
            ident_sb = res.tile([128, 128], F32)
            nc.sync.dma_start(out=ident_sb[:], in_=ident_d[:])
            xt1_sb = res.tile([D + 1, nw, 128], F32)
            nc.sync.dma_start(
                out=xt1_sb[:], in_=xt1_d.rearrange("f (w p) -> f w p", p=128))
            wcb_sb = res.tile([D + 1, HID], F32)
            nc.sync.dma_start(out=wcb_sb[:], in_=wcb_d[:])
            w1l_sb = res.tile([D, HID], F32)
            nc.sync.dma_start(out=w1l_sb[:], in_=w1l_d[:])
            w2lr_sb = res.tile([128, 2 * 2 * OUT], F32)
            nc.sync.dma_start(out=w2lr_sb[:], in_=w2lr_d[:])
            gamma_sb = res.tile([128, HID], F32)
            nc.sync.dma_start(out=gamma_sb[:], in_=gamma_d[:])
            beta_sb = res.tile([128, HID], F32)
            nc.sync.dma_start(out=beta_sb[:], in_=beta_d[:])
            b2_sb = res.tile([128, OUT], F32)
            nc.sync.dma_start(out=b2_sb[:], in_=b2_d[:])
            rc_sb = res.tile([128, nw], F32)
            nc.sync.dma_start(out=rc_sb[:], in_=rc_d[:])
            rcbc_sb = res.tile([128, nw, 128], F32)
            nc.sync.dma_start(
                out=rcbc_sb[:],
                in_=rcbc_d.rearrange("p (w j) -> p w j", j=128))
            pat4_sb = res.tile([4, 128], BF16)
            nc.sync.dma_start(out=pat4_sb[:], in_=pat4_d[:])

            h_sb = res.tile([128, nw, HID], F32)
            pr_sb = res.tile([128, nw, 2 * OUT], F32)
            out_sb = res.tile([128, nw, OUT], F32)
            pt4_sb = res.tile([4, cp], BF16)

            pt4_dram = dram.tile([4, cp], BF16)
            pt4all_dram = dram.tile([4 * N_CORES, cp], BF16)
            pb2_lo = dram.tile([half, 128], BF16)
            pb2_hi = dram.tile([half, 128], BF16)

            # ---- aggregation pass: edge stream -> per-window PSUM aggT
            def aggregation(tables, wout, sink):
                """tables: (lo, hi) DRAM [half, 128] bf16 with rows
                [v_hi (wout) | v_lo (wout) | ...]. For each window w calls
                sink(s, w, pw) where pw [wout, 128] PSUM holds the
                hi+lo-summed transposed aggregate (features x dst): two
                chained matmuls per chunk (lhsT = hi cols / lo cols of the
                gathered tile, rhs = dst one-hot) accumulate both parts.
                """
                with (
                    tc.tile_pool(name="gpool", bufs=3) as gpool,
                    tc.tile_pool(name="opool", bufs=2) as opool,
                    tc.tile_pool(name="pwpool", bufs=2, space="PSUM") as pwp,
                ):
                    qn = 0
                    for s in range(2):
                        table = tables[s]
                        rows = sched[s]
                        if not rows:
                            continue
                        c0 = rows[0][1]
                        c1 = rows[-1][2] + 1
                        gbufs = {}
                        for b0 in range(c0, c1, BATCH_CHUNKS):
                            b1 = min(b0 + BATCH_CHUNKS, c1)
                            g = gpool.tile([128, BATCH_CHUNKS, 128], BF16,
                                           tag="gbuf")
                            n_idx = (b1 - b0) * 128
                            nc.gpsimd.dma_gather(
                                out_ap=g[:, 0:b1 - b0, :],
                                in_ap=table,
                                idxs_ap=gidx_sb[:, b0 * 8:b1 * 8],
                                num_idxs=n_idx,
                                num_idxs_reg=n_idx,
                                elem_size=128,
                                single_packet=False,
                                queue_num=qn)
                            qn = (qn + 1) % NSWQ
                            gbufs[b0] = g
                        for w, first, last in rows:
                            pw = pwp.tile([wout, 128], F32, tag="pw")
                            for gci in range(first, last + 1):
                                b0 = c0 + ((gci - c0) // BATCH_CHUNKS) \
                                    * BATCH_CHUNKS
                                gb = gbufs[b0]
                                o = opool.tile([128, 128], BF16, tag="O")
                                nc.vector.tensor_scalar(
                                    out=o[:],
                                    in0=iota_sb[:],
                                    scalar1=dstf_sb[:, gci:gci + 1],
                                    scalar2=None,
                                    op0=mybir.AluOpType.is_equal)
                                nc.tensor.matmul(
                                    pw[:], gb[:, gci - b0, 0:wout], o[:],
                                    start=(gci == first), stop=False)
                                nc.tensor.matmul(
                                    pw[:], gb[:, gci - b0, wout:2 * wout],
                                    o[:],
                                    start=False, stop=(gci == last))
                            sink(s, w, pw)

            # conv1 aggT accumulator in SBUF: [128, nw, 128] f32:
            # rows 0:64 hi-part, 64:128 lo-part (summed over streams)
            aggT1 = res.tile([D, nw, 128], F32)
            nc.vector.memset(aggT1[:], 0.0)

            def sink1(s, w, pw):
                nc.vector.tensor_tensor(
                    out=aggT1[:, w, :], in0=aggT1[:, w, :], in1=pw[:],
                    op=mybir.AluOpType.add)

            aggregation((xhl_lo, xhl_hi), D, sink1)

            # ================= dense phase =================
            with (
                tc.tile_pool(name="dwork", bufs=2) as dwork,
                tc.tile_pool(name="dsmall", bufs=2) as dsmall,
                tc.tile_pool(name="dpsum", bufs=2, space="PSUM") as dpsum,
                tc.tile_pool(name="dpsum2", bufs=2, space="PSUM") as dpsum2,
            ):
                for n in range(nw):
                    # meanT = (aggT_hi + aggT_lo) * rc_bc   [64, 128]
                    meanT = dwork.tile([D, 128], F32, tag="meanT")
                    nc.vector.tensor_tensor(
                        out=meanT[:], in0=aggT1[:, n, :],
                        in1=rcbc_sb[0:D, n, :], op=mybir.AluOpType.mult)

                    # x1 = x@Wc + bc + mean@W1l   [128, HID] (fp32)
                    x1p = dpsum2.tile([128, HID], F32, tag="x1")
                    nc.tensor.matmul(x1p[:], xt1_sb[:, n, :], wcb_sb[:],
                                     start=True, stop=False)
                    nc.tensor.matmul(x1p[:], meanT[:], w1l_sb[:],
                                     start=False, stop=True)

                    # LayerNorm + ELU
                    mu = dsmall.tile([128, 1], F32, tag="mu")
                    nc.vector.reduce_sum(out=mu[:], in_=x1p[:],
                                         axis=mybir.AxisListType.X)
                    nc.vector.tensor_scalar(
                        out=mu[:], in0=mu[:], scalar1=1.0 / HID,
                        scalar2=None, op0=mybir.AluOpType.mult)
                    xc = dwork.tile([128, HID], F32, tag="xc")
                    nc.vector.tensor_scalar(
                        out=xc[:], in0=x1p[:], scalar1=mu[:], scalar2=None,
                        op0=mybir.AluOpType.subtract)
                    sq = dwork.tile([128, HID], F32, tag="sq")
                    var = dsmall.tile([128, 1], F32, tag="var")
                    nc.scalar.activation(
                        sq[:], xc[:], mybir.ActivationFunctionType.Square,
                        accum_out=var[:])
                    rstd = dsmall.tile([128, 1], F32, tag="rstd")
                    nc.vector.tensor_scalar(
                        out=rstd[:], in0=var[:], scalar1=1.0 / HID,
                        scalar2=LN_EPS, op0=mybir.AluOpType.mult,
                        op1=mybir.AluOpType.add)
                    nc.scalar.activation(
                        rstd[:], rstd[:], mybir.ActivationFunctionType.Sqrt)
                    nc.vector.reciprocal(rstd[:], rstd[:])
                    y = dwork.tile([128, HID], F32, tag="y")
                    nc.scalar.activation(
                        y[:], xc[:], mybir.ActivationFunctionType.Copy,
                        scale=rstd[:])
                    nc.vector.tensor_tensor(
                        out=y[:], in0=y[:], in1=gamma_sb[:],
                        op=mybir.AluOpType.mult)
                    nc.vector.tensor_tensor(
                        out=y[:], in0=y[:], in1=beta_sb[:],
                        op=mybir.AluOpType.add)
                    # ELU: h = max(y,0)-1 + exp(min(y,0))
                    m0 = dwork.tile([128, HID], F32, tag="m0")
                    nc.vector.tensor_scalar(
                        out=m0[:], in0=y[:], scalar1=0.0, scalar2=None,
                        op0=mybir.AluOpType.min)
                    ex = dwork.tile([128, HID], F32, tag="ex")
                    nc.scalar.activation(
                        ex[:], m0[:], mybir.ActivationFunctionType.Exp)
                    rm1 = dwork.tile([128, HID], F32, tag="rm1")
                    nc.vector.tensor_scalar(
                        out=rm1[:], in0=y[:], scalar1=0.0, scalar2=-1.0,
                        op0=mybir.AluOpType.max, op1=mybir.AluOpType.add)
                    nc.vector.tensor_tensor(
                        out=h_sb[:, n, :], in0=rm1[:], in1=ex[:],
                        op=mybir.AluOpType.add)

                    # pr = h @ [W2l | W2r]  [128, 4] fp32
                    prp = dpsum2.tile([128, 2 * OUT], F32, tag="pr")
                    for hh in range(2):
                        tph = dpsum.tile([128, 128], F32, tag="tph")
                        nc.tensor.transpose(
                            tph[:], h_sb[:, n, 128 * hh:128 * (hh + 1)],
                            ident_sb[:])
                        hts = dwork.tile([128, 128], F32, tag="hts")
                        nc.scalar.activation(
                            hts[:], tph[:],
                            mybir.ActivationFunctionType.Copy)
                        nc.tensor.matmul(
                            prp[:], hts[:],
                            w2lr_sb[:, 4 * hh:4 * (hh + 1)],
                            start=(hh == 0), stop=(hh == 1))
                    nc.scalar.activation(
                        pr_sb[:, n, :], prp[:],
                        mybir.ActivationFunctionType.Copy)

                    # p4 = [p_hi (2) | p_lo (2)] from p = pr[:, 0:2]
                    p_hi_b = dsmall.tile([128, OUT], BF16, tag="phb")
                    nc.vector.tensor_copy(p_hi_b[:], pr_sb[:, n, 0:OUT])
                    p_hi_f = dsmall.tile([128, OUT], F32, tag="phf")
                    nc.vector.tensor_copy(p_hi_f[:], p_hi_b[:])
                    p4 = dwork.tile([128, 2 * OUT], F32, tag="p4")
                    nc.vector.tensor_copy(p4[:, 0:OUT], p_hi_f[:])
                    nc.vector.tensor_tensor(
                        out=p4[:, OUT:2 * OUT], in0=pr_sb[:, n, 0:OUT],
                        in1=p_hi_f[:], op=mybir.AluOpType.subtract)
                    # pt4[:, w*128:...] = p4^T
                    ptp = dpsum.tile([2 * OUT, 128], F32, tag="ptp")
                    nc.tensor.transpose(ptp[:], p4[:], ident_sb[:])
                    nc.scalar.activation(
                        pt4_sb[:, 128 * n:128 * (n + 1)], ptp[:],
                        mybir.ActivationFunctionType.Copy)

                nc.sync.dma_start(out=pt4_dram[:], in_=pt4_sb[:])

            # ================= p4 all-gather =================
            nc.gpsimd.collective_compute(
                "AllGather",
                mybir.AluOpType.bypass,
                replica_groups=[list(range(N_CORES))],
                ins=[pt4_dram.opt()],
                outs=[pt4all_dram.opt()],
            )

            # ============ build pb2 (replicated p4 table, bf16) ============
            n_glob = N_CORES * nw
            with (
                tc.tile_pool(name="bstage", bufs=3) as bstage,
                tc.tile_pool(name="bpt", bufs=1) as bpt,
                tc.tile_pool(name="bpsum", bufs=2, space="PSUM") as bpsum,
            ):
                stage_n = 7  # 49 % 7 == 0; half_w = 196 = 28*7
                stage = None
                ptb = None
                for j in range(n_glob):
                    c = j // nw
                    jw = j % nw
                    if jw == 0:
                        ptb = bpt.tile([4, cp], BF16, tag="ptb", name="ptb")
                        nc.sync.dma_start(
                            out=ptb[:], in_=pt4all_dram[4 * c:4 * c + 4, :])
                    pp = bpsum.tile([128, 128], F32, tag="pb2p")
                    nc.tensor.matmul(
                        pp[:],
                        ptb[:, 128 * jw:128 * (jw + 1)],
                        pat4_sb[:], start=True, stop=True)
                    if j % stage_n == 0:
                        stage = bstage.tile([128, stage_n, 128], BF16,
                                            tag="stage")
                    nc.scalar.activation(
                        stage[:, j % stage_n, :], pp[:],
                        mybir.ActivationFunctionType.Copy)
                    if j % stage_n == stage_n - 1:
                        j0 = j - stage_n + 1
                        r0 = j0 * 128
                        if r0 < half:
                            dst = pb2_lo[r0:r0 + stage_n * 128, :]
                        else:
                            dst = pb2_hi[r0 - half:r0 - half
                                         + stage_n * 128, :]
                        nc.sync.dma_start(
                            out=dst.rearrange("(s p) d -> p s d", p=128),
                            in_=stage[:])

            # ================= conv2 aggregation =================
            agg2T = res.tile([OUT, nw, 128], F32)
            nc.vector.memset(agg2T[:], 0.0)

            def sink2(s, w, pw):
                nc.vector.tensor_tensor(
                    out=agg2T[:, w, :], in0=agg2T[:, w, :],
                    in1=pw[:], op=mybir.AluOpType.add)

            aggregation((pb2_lo, pb2_hi), OUT, sink2)

            # ================= output =================
            # out[dst, c] = (agg2_hi + agg2_lo)[c, dst] * rc + r + b2
            with (
                tc.tile_pool(name="fsmall", bufs=2) as fsmall,
                tc.tile_pool(name="fpsum", bufs=2, space="PSUM") as fpsum,
            ):
                for n in range(nw):
                    # transpose [2, 128] -> [128, 2]
                    a2t = fpsum.tile([128, OUT], F32, tag="a2t")
                    nc.tensor.transpose(a2t[:], agg2T[:, n, :],
                                        ident_sb[0:OUT, 0:OUT])
                    t = fsmall.tile([128, OUT], F32, tag="fo")
                    nc.vector.tensor_scalar(
                        out=t[:], in0=a2t[:], scalar1=rc_sb[:, n:n + 1],
                        scalar2=None, op0=mybir.AluOpType.mult)
                    nc.vector.tensor_tensor(
                        out=t[:], in0=t[:], in1=pr_sb[:, n, OUT:2 * OUT],
                        op=mybir.AluOpType.add)
                    nc.vector.tensor_tensor(
                        out=out_sb[:, n, :], in0=t[:], in1=b2_sb[:],
                        op=mybir.AluOpType.add)
            nc.sync.dma_start(
                out=out_d.rearrange("(w p) c -> p w c", p=128),
                in_=out_sb[:])

    nc.compile()
    return nc


def make_inputs(plan, x, W1l, W1r, b1, Wskip, bskip, gamma, beta, W2l, W2r,
                b2, n_nodes):
    import ml_dtypes
    cp, half, npad, nw = plan["cp"], plan["half"], plan["npad"], plan["nw"]
    xp = np.zeros((npad, D), np.float32)
    xp[:n_nodes] = np.asarray(x, np.float32)
    x_hi = xp.astype(ml_dtypes.bfloat16)
    x_lo = (xp - x_hi.astype(np.float32)).astype(ml_dtypes.bfloat16)
    xhl = np.concatenate([x_hi, x_lo], axis=1)  # [npad, 128] bf16

    wc = np.asarray(W1r, np.float32) + np.asarray(Wskip, np.float32)
    bc = np.asarray(b1, np.float32) + np.asarray(bskip, np.float32)
    wcb = np.concatenate([wc, bc[None, :]], axis=0)
    w2lr_full = np.concatenate(
        [np.asarray(W2l, np.float32), np.asarray(W2r, np.float32)], axis=1
    )  # [HID, 4]
    w2lr = (
        w2lr_full.reshape(2, 128, 2 * OUT).transpose(1, 0, 2)
        .reshape(128, 2 * 2 * OUT).copy()
    )
    iota = np.tile(np.arange(128, dtype=np.float32)[None, :], (128, 1))
    ident = np.eye(128, dtype=np.float32)
    # pat4[q, 128 cols] = 1 where col % 4 == q
    import ml_dtypes as _md
    pat4 = np.zeros((4, 128), np.float32)
    for q in range(4):
        pat4[q, q::4] = 1.0
    pat4 = pat4.astype(_md.bfloat16)
    gamma_bc = np.tile(np.asarray(gamma, np.float32)[None, :], (128, 1))
    beta_bc = np.tile(np.asarray(beta, np.float32)[None, :], (128, 1))
    b2_bc = np.tile(np.asarray(b2, np.float32)[None, :], (128, 1))

    common = dict(
        xhl_lo=xhl[:half].copy(), xhl_hi=xhl[half:].copy(),
        iota=iota, ident=ident,
        wcb=wcb, w1l=np.asarray(W1l, np.float32), w2lr=w2lr,
        gamma_bc=gamma_bc, beta_bc=beta_bc, b2_bc=b2_bc, pat4=pat4,
    )
    in_maps = []
    for c in range(N_CORES):
        m = dict(common)
        xt = xp[cp * c:cp * (c + 1)].T.copy()          # [64, cp]
        xt1 = np.concatenate([xt, np.ones((1, cp), np.float32)], axis=0)
        m["xt1"] = xt1
        m["gidx"] = plan["gidx_tile"][c]
        m["dstf"] = plan["dstf_tile"][c]
        m["rc"] = plan["rc_tile"][c]
        m["rc_bc"] = plan["rc_bc"][c]
        in_maps.append(m)
    return in_maps


_CACHE = {}


def _get_compiled(edge_index, n_nodes):
    key = (edge_index.tobytes()[:512], edge_index.shape, n_nodes)
    if key not in _CACHE:
        plan = make_plan(edge_index, n_nodes)
        nc = build_program(plan)
        _CACHE[key] = (plan, nc)
    return _CACHE[key]


def run(inputs, trace=False):
    x = np.asarray(inputs["x"], np.float32)
    edge_index = np.asarray(inputs["edge_index"], np.int32)
    n_nodes = x.shape[0]
    plan, nc = _get_compiled(edge_index, n_nodes)
    in_maps = make_inputs(
        plan, x, inputs["W1l"], inputs["W1r"], inputs["b1"], inputs["Wskip"],
        inputs["bskip"], inputs["gamma"], inputs["beta"], inputs["W2l"],
        inputs["W2r"], inputs["b2"], n_nodes)
    res = run_bass_kernel_spmd(
        nc, in_maps, list(range(N_CORES)), trace=trace)
    cp = plan["cp"]
    out = np.empty((n_nodes, OUT), np.float32)
    for c in range(N_CORES):
        lo = cp * c
        hi = min(cp * (c + 1), n_nodes)
        out[lo:hi] = res.results[c]["out"][0:hi - lo]
    return out, res


def kernel(**inputs) -> np.ndarray:
    out, _ = run(inputs)
    return out


# revision 21
# speedup vs baseline: 1.5359x; 1.3304x over previous
"""GeneSAGE (2-layer GraphSAGE + skip + LayerNorm + ELU) on 8 Trainium2 cores.

V2 design. Edge-parallel by destination range: core c owns dst nodes
[cp*c, cp*(c+1)), cp=6272. Edges bucketed host-side by (dst-core, src-half,
dst-window) into 128-edge chunks with an SPMD-common chunk schedule.

Key speed choices (measured on HW):
- Gathered x is stored hi/lo bf16 split ([bf16(x) | bf16(x-bf16(x))] 256B
  rows) so the aggregation matmul runs in bf16 (1 instr, ~130ns) while
  keeping fp32-class accuracy (products exact vs 0/1 one-hot; PSUM fp32).
- One-hot built on DVE via is_equal (bf16 out, fp32 iota/scalar) ~160ns.
- Aggregation matmul operand-swapped: lhsT = gathered tile, rhs = one-hot
  -> PSUM holds aggT (feature-major), killing the dense-phase transposes.
- Edge counts (degrees) precomputed on host; no count matmuls.
- dma_gather calls round-robin over 2 SWDGE queues: descriptor generation
  of the next call overlaps the wait of the current (7.9 -> ~4.7 ns/idx).
- conv2 gathers a [p_hi | p_lo]-replicated 256B-row table (built on device
  from an AllGather of p^T hi/lo parts); one bf16 matmul per chunk.
"""

import numpy as np

import concourse.mybir as mybir
from concourse import bacc, bass, tile
from concourse.bass_utils import run_bass_kernel_spmd

F32 = mybir.dt.float32
BF16 = mybir.dt.bfloat16
I16 = mybir.dt.int16

N_CORES = 8
D = 64          # input feature dim
HID = 256
OUT = 2
LN_EPS = 1e-5
BATCH_CHUNKS = 32   # chunks per dma_gather call
NSWQ = 2            # SWDGE queues (2 parallel descriptor-gen cores)


def make_plan(edge_index: np.ndarray, n_nodes: int):
    """Host-side edge bucketing + degree counts. SPMD-common schedule."""
    cp = int(np.ceil(n_nodes / (N_CORES * 128))) * 128
    nw = cp // 128
    npad = N_CORES * cp
    half = npad // 2
    assert half <= 32768, "int16 gather index limit"

    src = edge_index[0].astype(np.int64)
    dst = edge_index[1].astype(np.int64)
    E = src.shape[0]

    core = dst // cp
    stream = (src >= half).astype(np.int64)
    win = (dst % cp) // 128
    ngrp_per_core = 2 * nw
    key = (core * 2 + stream) * nw + win
    order = np.argsort(key, kind="stable")
    counts = np.bincount(key, minlength=N_CORES * ngrp_per_core).reshape(
        N_CORES, 2, nw
    )
    nchunks = -(-counts.max(axis=0) // 128)  # [2, nw]
    off = np.zeros((2, nw), np.int64)
    running = 0
    for s in range(2):
        for w in range(nw):
            off[s, w] = running
            running += nchunks[s, w]
    c_total = int(running)
    e_slots = c_total * 128

    sk = key[order]
    grp_start = np.searchsorted(sk, np.arange(N_CORES * ngrp_per_core))
    rank = np.arange(E) - grp_start[sk]
    s_of = (sk // nw) % 2
    w_of = sk % nw
    c_of = sk // ngrp_per_core
    slot = off[s_of, w_of] * 128 + rank

    gidx = np.zeros((N_CORES, e_slots), np.int16)
    dstf = np.full((N_CORES, e_slots), -1.0, np.float32)
    gidx[c_of, slot] = (src[order] - s_of * half).astype(np.int16)
    dstf[c_of, slot] = (dst[order] % cp - w_of * 128).astype(np.float32)

    # gather idx tile [128, e_slots//16*8]: tile[p, j] = gidx[16*j + p%16]
    a = gidx.reshape(N_CORES, e_slots // 16, 16).transpose(0, 2, 1)
    gidx_tile = np.tile(a, (1, 8, 1)).copy()  # [c, 128, J]
    dstf_tile = (
        dstf.reshape(N_CORES, c_total, 128).transpose(0, 2, 1).copy()
    )  # [c, 128, C]

    sched = []
    for s in range(2):
        rows = []
        for w in range(nw):
            n = int(nchunks[s, w])
            if n == 0:
                continue
            first = int(off[s, w])
            rows.append((w, first, first + n - 1))
        sched.append(rows)

    # degree counts per node (for mean); reciprocal, per-core window tiles
    cnt = np.bincount(dst, minlength=npad).astype(np.float32)
    rc = 1.0 / np.maximum(cnt, 1.0)
    # rc_tile[c][p, w] = rc[cp*c + 128*w + p]
    rc_tile = rc.reshape(N_CORES, nw, 128).transpose(0, 2, 1).copy()
    # rc broadcast tile for aggT scaling: [128, cp] where [p, 128*w+j] =
    # rc[cp*c + 128*w + j]  (same value down all partitions)
    rc_bc = np.broadcast_to(
        rc.reshape(N_CORES, 1, cp), (N_CORES, 128, cp)
    ).copy()

    return dict(
        cp=cp, nw=nw, npad=npad, half=half,
        c_total=c_total,
        sched=sched,
        gidx_tile=gidx_tile, dstf_tile=dstf_tile,
        rc_tile=rc_tile, rc_bc=rc_bc,
    )


def build_program(plan):
    cp, nw, half = plan["cp"], plan["nw"], plan["half"]
    c_total = plan["c_total"]
    sched = plan["sched"]
    J = c_total * 8
    half_w = half // 128  # 128-row blocks per half table

    nc = bacc.Bacc("TRN2", target_bir_lowering=False, debug=False,
                   num_devices=N_CORES, num_swdge_queues=NSWQ)

    def inp(name, shape, dt=F32):
        return nc.dram_tensor(name, shape, dt, kind="ExternalInput").ap()

    xhl_lo = inp("xhl_lo", [half, 128], BF16)   # [x_hi | x_lo] rows
    xhl_hi = inp("xhl_hi", [half, 128], BF16)
    gidx_d = inp("gidx", [128, J], I16)
    dstf_d = inp("dstf", [128, c_total])
    iota_d = inp("iota", [128, 128])
    xt1_d = inp("xt1", [D + 1, cp])             # x_loc^T with ones row
    wcb_d = inp("wcb", [D + 1, HID])            # [Wc; bc] fp32
    w1l2_d = inp("w1l2", [2 * D, HID])
    w2lr_d = inp("w2lr", [128, 2 * 2 * OUT])    # packed halves [W2l|W2r]
    ident_d = inp("ident", [128, 128])
    gamma_d = inp("gamma_bc", [128, HID])
    beta_d = inp("beta_bc", [128, HID])
    b2_d = inp("b2_bc", [128, OUT])
    rc_d = inp("rc", [128, nw])
    pat4_d = inp("pat4", [4, 128], BF16)              # p4 tiling pattern
    out_d = nc.dram_tensor("out", [cp, OUT], F32, kind="ExternalOutput").ap()

    with tile.TileContext(nc) as tc:
        with (
            tc.tile_pool(name="res", bufs=1) as res,
            tc.tile_pool(name="dram", bufs=1, space="DRAM") as dram,
        ):
            # ---- resident tiles
            gidx_sb = res.tile([128, J], I16)
            nc.sync.dma_start(out=gidx_sb[:], in_=gidx_d[:])
            dstf_sb = res.tile([128, c_total], F32)
            nc.sync.dma_start(out=dstf_sb[:], in_=dstf_d[:])
            iota_sb = res.tile([128, 128], F32)
            nc.sync.dma_start(out=iota_sb[:], in_=iota_d[:])
            ident_sb = res.tile([128, 128], F32)
            nc.sync.dma_start(out=ident_sb[:], in_=ident_d[:])
            xt1_sb = res.tile([D + 1, nw, 128], F32)
            nc.sync.dma_start(
                out=xt1_sb[:], in_=xt1_d.rearrange("f (w p) -> f w p", p=128))
            wcb_sb = res.tile([D + 1, HID], F32)
            nc.sync.dma_start(out=wcb_sb[:], in_=wcb_d[:])
            w1l2_sb = res.tile([2 * D, HID], F32)
            nc.sync.dma_start(out=w1l2_sb[:], in_=w1l2_d[:])
            w2lr_sb = res.tile([128, 2 * 2 * OUT], F32)
            nc.sync.dma_start(out=w2lr_sb[:], in_=w2lr_d[:])
            gamma_sb = res.tile([128, HID], F32)
            nc.sync.dma_start(out=gamma_sb[:], in_=gamma_d[:])
            beta_sb = res.tile([128, HID], F32)
            nc.sync.dma_start(out=beta_sb[:], in_=beta_d[:])
            b2_sb = res.tile([128, OUT], F32)
            nc.sync.dma_start(out=b2_sb[:], in_=b2_d[:])
            rc_sb = res.tile([128, nw], F32)
            nc.sync.dma_start(out=rc_sb[:], in_=rc_d[:])
            pat4_sb = res.tile([4, 128], BF16)
            nc.sync.dma_start(out=pat4_sb[:], in_=pat4_d[:])

            h_sb = res.tile([128, nw, HID], F32)
            pr_sb = res.tile([128, nw, 2 * OUT], F32)
            out_sb = res.tile([128, nw, OUT], F32)
            pt4_sb = res.tile([4, cp], BF16)

            pt4_dram = dram.tile([4, cp], BF16)
            pt4all_dram = dram.tile([4 * N_CORES, cp], BF16)
            pb2_lo = dram.tile([half, 128], BF16)
            pb2_hi = dram.tile([half, 128], BF16)

            # ---- aggregation pass: edge stream -> per-window PSUM aggT
            def aggregation(tables, wout, sink):
                """tables: (lo, hi) DRAM [half, 128] bf16 with rows
                [v_hi | v_lo | ...] (wout cols used per chunk matmul).
                For each window w calls sink(s, w, pw) with pw [wout, 128]
                PSUM = transposed aggregate over the window's chunks (one
                bf16 matmul per chunk: lhsT = gathered cols 0:wout, rhs =
                dst one-hot). hi/lo parts are NOT summed here; the
                consumer's contraction or a later transpose handles it.
                iota lives in PSUM so the is_eq runs in 1-port mode and
                does not contend with SWDGE descriptor traffic.
                """
                with (
                    tc.tile_pool(name="gpool", bufs=3) as gpool,
                    tc.tile_pool(name="opool", bufs=2) as opool,
                    tc.tile_pool(name="pwpool", bufs=2, space="PSUM") as pwp,
                    tc.tile_pool(name="ipool", bufs=1, space="PSUM") as ipl,
                ):
                    iota_psum = ipl.tile([128, 128], F32, tag="iop")
                    nc.scalar.activation(
                        iota_psum[:], iota_sb[:],
                        mybir.ActivationFunctionType.Copy)
                    qn = 0
                    for s in range(2):
                        table = tables[s]
                        rows = sched[s]
                        if not rows:
                            continue
                        c0 = rows[0][1]
                        c1 = rows[-1][2] + 1
                        gbufs = {}
                        for b0 in range(c0, c1, BATCH_CHUNKS):
                            b1 = min(b0 + BATCH_CHUNKS, c1)
                            g = gpool.tile([128, BATCH_CHUNKS, 128], BF16,
                                           tag="gbuf")
                            n_idx = (b1 - b0) * 128
                            nc.gpsimd.dma_gather(
                                out_ap=g[:, 0:b1 - b0, :],
                                in_ap=table,
                                idxs_ap=gidx_sb[:, b0 * 8:b1 * 8],
                                num_idxs=n_idx,
                                num_idxs_reg=n_idx,
                                elem_size=128,
                                single_packet=False,
                                queue_num=qn)
                            qn = (qn + 1) % NSWQ
                            gbufs[b0] = g
                        for w, first, last in rows:
                            pw = pwp.tile([wout, 128], F32, tag="pw")
                            for gci in range(first, last + 1):
                                b0 = c0 + ((gci - c0) // BATCH_CHUNKS) \
                                    * BATCH_CHUNKS
                                gb = gbufs[b0]
                                o = opool.tile([128, 128], BF16, tag="O")
                                nc.vector.tensor_scalar(
                                    out=o[:],
                                    in0=iota_psum[:],
                                    scalar1=dstf_sb[:, gci:gci + 1],
                                    scalar2=None,
                                    op0=mybir.AluOpType.is_equal)
                                nc.tensor.matmul(
                                    pw[:], gb[:, gci - b0, 0:wout], o[:],
                                    start=(gci == first), stop=(gci == last))
                            sink(s, w, pw)

            # conv1 aggT accumulator in SBUF: [128, nw, 128] f32:
            # rows 0:64 hi-part, 64:128 lo-part (summed over streams);
            # the dense matmul against [W1l; W1l] sums hi+lo via its
            # contraction.
            aggT1 = res.tile([128, nw, 128], F32)
            nc.vector.memset(aggT1[:], 0.0)

            # per-window last stream holding chunks (to fire the dense /
            # output phase as soon as that window's aggregate completes,
            # overlapping it with the remaining gather stream)
            last_s = {}
            for s_ in range(2):
                for (w_, _f, _l) in sched[s_]:
                    last_s[w_] = s_

            # ================= conv1 + interleaved dense =================
            with (
                tc.tile_pool(name="dwork", bufs=2) as dwork,
                tc.tile_pool(name="dsmall", bufs=2) as dsmall,
                tc.tile_pool(name="dpsum", bufs=1, space="PSUM") as dpsum,
                tc.tile_pool(name="dpsum2", bufs=1, space="PSUM") as dpsum2,
            ):
                def dense_window(n):
                    # x1 = x@Wc + bc + ((aggT_hi+aggT_lo)@W1l) * rc
                    # the [W1l; W1l] contraction over the 128 stacked
                    # hi/lo rows sums the two parts for free
                    x1p = dpsum2.tile([128, HID], F32, tag="x1")
                    nc.tensor.matmul(x1p[:], xt1_sb[:, n, :], wcb_sb[:],
                                     start=True, stop=True)
                    x1m = dpsum2.tile([128, HID], F32, tag="x1m")
                    nc.tensor.matmul(x1m[:], aggT1[:, n, :], w1l2_sb[:],
                                     start=True, stop=True)
                    xs = dwork.tile([128, HID], F32, tag="xs")
                    nc.scalar.activation(
                        xs[:], x1m[:], mybir.ActivationFunctionType.Copy,
                        scale=rc_sb[:, n:n + 1])
                    x1f = dwork.tile([128, HID], F32, tag="x1f")
                    nc.vector.tensor_tensor(
                        out=x1f[:], in0=xs[:], in1=x1p[:],
                        op=mybir.AluOpType.add)

                    # LayerNorm + ELU
                    mu = dsmall.tile([128, 1], F32, tag="mu")
                    nc.vector.reduce_sum(out=mu[:], in_=x1f[:],
                                         axis=mybir.AxisListType.X)
                    nc.vector.tensor_scalar(
                        out=mu[:], in0=mu[:], scalar1=1.0 / HID,
                        scalar2=None, op0=mybir.AluOpType.mult)
                    xc = dwork.tile([128, HID], F32, tag="xc")
                    nc.vector.tensor_scalar(
                        out=xc[:], in0=x1f[:], scalar1=mu[:], scalar2=None,
                        op0=mybir.AluOpType.subtract)
                    sq = dwork.tile([128, HID], F32, tag="sq")
                    var = dsmall.tile([128, 1], F32, tag="var")
                    nc.scalar.activation(
                        sq[:], xc[:], mybir.ActivationFunctionType.Square,
                        accum_out=var[:])
                    rstd = dsmall.tile([128, 1], F32, tag="rstd")
                    nc.vector.tensor_scalar(
                        out=rstd[:], in0=var[:], scalar1=1.0 / HID,
                        scalar2=LN_EPS, op0=mybir.AluOpType.mult,
                        op1=mybir.AluOpType.add)
                    nc.scalar.activation(
                        rstd[:], rstd[:], mybir.ActivationFunctionType.Sqrt)
                    nc.vector.reciprocal(rstd[:], rstd[:])
                    y = dwork.tile([128, HID], F32, tag="y")
                    nc.scalar.activation(
                        y[:], xc[:], mybir.ActivationFunctionType.Copy,
                        scale=rstd[:])
                    nc.vector.tensor_tensor(
                        out=y[:], in0=y[:], in1=gamma_sb[:],
                        op=mybir.AluOpType.mult)
                    nc.vector.tensor_tensor(
                        out=y[:], in0=y[:], in1=beta_sb[:],
                        op=mybir.AluOpType.add)
                    # ELU: h = max(y,0)-1 + exp(min(y,0))
                    m0 = dwork.tile([128, HID], F32, tag="m0")
                    nc.vector.tensor_scalar(
                        out=m0[:], in0=y[:], scalar1=0.0, scalar2=None,
                        op0=mybir.AluOpType.min)
                    ex = dwork.tile([128, HID], F32, tag="ex")
                    nc.scalar.activation(
                        ex[:], m0[:], mybir.ActivationFunctionType.Exp)
                    rm1 = dwork.tile([128, HID], F32, tag="rm1")
                    nc.vector.tensor_scalar(
                        out=rm1[:], in0=y[:], scalar1=0.0, scalar2=-1.0,
                        op0=mybir.AluOpType.max, op1=mybir.AluOpType.add)
                    nc.vector.tensor_tensor(
                        out=h_sb[:, n, :], in0=rm1[:], in1=ex[:],
                        op=mybir.AluOpType.add)

                    # pr = h @ [W2l | W2r]  [128, 4] fp32
                    prp = dpsum2.tile([128, 2 * OUT], F32, tag="pr")
                    for hh in range(2):
                        tph = dpsum.tile([128, 128], F32, tag="tph")
                        nc.tensor.transpose(
                            tph[:], h_sb[:, n, 128 * hh:128 * (hh + 1)],
                            ident_sb[:])
                        hts = dwork.tile([128, 128], F32, tag="hts")
                        nc.scalar.activation(
                            hts[:], tph[:],
                            mybir.ActivationFunctionType.Copy)
                        nc.tensor.matmul(
                            prp[:], hts[:],
                            w2lr_sb[:, 4 * hh:4 * (hh + 1)],
                            start=(hh == 0), stop=(hh == 1))
                    nc.scalar.activation(
                        pr_sb[:, n, :], prp[:],
                        mybir.ActivationFunctionType.Copy)

                    # p4 = [p_hi (2) | p_lo (2)] from p = pr[:, 0:2]
                    p_hi_b = dsmall.tile([128, OUT], BF16, tag="phb")
                    nc.vector.tensor_copy(p_hi_b[:], pr_sb[:, n, 0:OUT])
                    p_hi_f = dsmall.tile([128, OUT], F32, tag="phf")
                    nc.vector.tensor_copy(p_hi_f[:], p_hi_b[:])
                    p4 = dwork.tile([128, 2 * OUT], F32, tag="p4")
                    nc.vector.tensor_copy(p4[:, 0:OUT], p_hi_f[:])
                    nc.vector.tensor_tensor(
                        out=p4[:, OUT:2 * OUT], in0=pr_sb[:, n, 0:OUT],
                        in1=p_hi_f[:], op=mybir.AluOpType.subtract)
                    # pt4[:, w*128:...] = p4^T
                    ptp = dpsum.tile([2 * OUT, 128], F32, tag="ptp")
                    nc.tensor.transpose(ptp[:], p4[:], ident_sb[:])
                    nc.scalar.activation(
                        pt4_sb[:, 128 * n:128 * (n + 1)], ptp[:],
                        mybir.ActivationFunctionType.Copy)

                def sink1(s, w, pw):
                    nc.vector.tensor_tensor(
                        out=aggT1[:, w, :], in0=aggT1[:, w, :], in1=pw[:],
                        op=mybir.AluOpType.add)
                    if last_s.get(w) == s:
                        dense_window(w)

                aggregation((xhl_lo, xhl_hi), 128, sink1)
                for n in range(nw):
                    if n not in last_s:
                        dense_window(n)

                nc.sync.dma_start(out=pt4_dram[:], in_=pt4_sb[:])

            # ================= p4 all-gather =================
            nc.gpsimd.collective_compute(
                "AllGather",
                mybir.AluOpType.bypass,
                replica_groups=[list(range(N_CORES))],
                ins=[pt4_dram.opt()],
                outs=[pt4all_dram.opt()],
            )

            # ============ build pb2 (replicated p4 table, bf16) ============
            n_glob = N_CORES * nw
            with (
                tc.tile_pool(name="bstage", bufs=3) as bstage,
                tc.tile_pool(name="bpt", bufs=1) as bpt,
                tc.tile_pool(name="bpsum", bufs=2, space="PSUM") as bpsum,
            ):
                stage_n = 7  # 49 % 7 == 0; half_w = 196 = 28*7
                stage = None
                ptb = None
                for j in range(n_glob):
                    c = j // nw
                    jw = j % nw
                    if jw == 0:
                        ptb = bpt.tile([4, cp], BF16, tag="ptb", name="ptb")
                        nc.sync.dma_start(
                            out=ptb[:], in_=pt4all_dram[4 * c:4 * c + 4, :])
                    pp = bpsum.tile([128, 128], F32, tag="pb2p")
                    nc.tensor.matmul(
                        pp[:],
                        ptb[:, 128 * jw:128 * (jw + 1)],
                        pat4_sb[:], start=True, stop=True)
                    if j % stage_n == 0:
                        stage = bstage.tile([128, stage_n, 128], BF16,
                                            tag="stage")
                    nc.scalar.activation(
                        stage[:, j % stage_n, :], pp[:],
                        mybir.ActivationFunctionType.Copy)
                    if j % stage_n == stage_n - 1:
                        j0 = j - stage_n + 1
                        r0 = j0 * 128
                        if r0 < half:
                            dst = pb2_lo[r0:r0 + stage_n * 128, :]
                        else:
                            dst = pb2_hi[r0 - half:r0 - half
                                         + stage_n * 128, :]
                        nc.sync.dma_start(
                            out=dst.rearrange("(s p) d -> p s d", p=128),
                            in_=stage[:])

            # ============ conv2 aggregation + interleaved output ============
            # out[dst, c] = (agg2_hi + agg2_lo)[c, dst] * rc + r + b2
            agg2T = res.tile([2 * OUT, nw, 128], F32)
            nc.vector.memset(agg2T[:], 0.0)

            with (
                tc.tile_pool(name="fsmall", bufs=2) as fsmall,
                tc.tile_pool(name="fpsum", bufs=2, space="PSUM") as fpsum,
            ):
                def out_window(n):
                    # transpose [4, 128] -> [128, 4]
                    a2t = fpsum.tile([128, 2 * OUT], F32, tag="a2t")
                    nc.tensor.transpose(a2t[:], agg2T[:, n, :],
                                        ident_sb[0:2 * OUT, 0:2 * OUT])
                    a4s = fsmall.tile([128, 2 * OUT], F32, tag="a4s")
                    nc.scalar.activation(
                        a4s[:], a2t[:], mybir.ActivationFunctionType.Copy)
                    asum = fsmall.tile([128, OUT], F32, tag="as")
                    nc.vector.tensor_tensor(
                        out=asum[:], in0=a4s[:, 0:OUT],
                        in1=a4s[:, OUT:2 * OUT], op=mybir.AluOpType.add)
                    t = fsmall.tile([128, OUT], F32, tag="fo")
                    nc.vector.tensor_scalar(
                        out=t[:], in0=asum[:], scalar1=rc_sb[:, n:n + 1],
                        scalar2=None, op0=mybir.AluOpType.mult)
                    nc.vector.tensor_tensor(
                        out=t[:], in0=t[:], in1=pr_sb[:, n, OUT:2 * OUT],
                        op=mybir.AluOpType.add)
                    nc.vector.tensor_tensor(
                        out=out_sb[:, n, :], in0=t[:], in1=b2_sb[:],
                        op=mybir.AluOpType.add)

                def sink2(s, w, pw):
                    nc.vector.tensor_tensor(
                        out=agg2T[:, w, :], in0=agg2T[:, w, :],
                        in1=pw[:], op=mybir.AluOpType.add)
                    if last_s.get(w) == s:
                        out_window(w)

                aggregation((pb2_lo, pb2_hi), 2 * OUT, sink2)
                for n in range(nw):
                    if n not in last_s:
                        out_window(n)
            nc.sync.dma_start(
                out=out_d.rearrange("(w p) c -> p w c", p=128),
                in_=out_sb[:])

    nc.compile()
    return nc


def make_inputs(plan, x, W1l, W1r, b1, Wskip, bskip, gamma, beta, W2l, W2r,
                b2, n_nodes):
    import ml_dtypes
    cp, half, npad, nw = plan["cp"], plan["half"], plan["npad"], plan["nw"]
    xp = np.zeros((npad, D), np.float32)
    xp[:n_nodes] = np.asarray(x, np.float32)
    x_hi = xp.astype(ml_dtypes.bfloat16)
    x_lo = (xp - x_hi.astype(np.float32)).astype(ml_dtypes.bfloat16)
    xhl = np.concatenate([x_hi, x_lo], axis=1)  # [npad, 128] bf16

    wc = np.asarray(W1r, np.float32) + np.asarray(Wskip, np.float32)
    bc = np.asarray(b1, np.float32) + np.asarray(bskip, np.float32)
    wcb = np.concatenate([wc, bc[None, :]], axis=0)
    w2lr_full = np.concatenate(
        [np.asarray(W2l, np.float32), np.asarray(W2r, np.float32)], axis=1
    )  # [HID, 4]
    w2lr = (
        w2lr_full.reshape(2, 128, 2 * OUT).transpose(1, 0, 2)
        .reshape(128, 2 * 2 * OUT).copy()
    )
    iota = np.tile(np.arange(128, dtype=np.float32)[None, :], (128, 1))
    ident = np.eye(128, dtype=np.float32)
    # pat4[q, 128 cols] = 1 where col % 4 == q
    import ml_dtypes as _md
    pat4 = np.zeros((4, 128), np.float32)
    for q in range(4):
        pat4[q, q::4] = 1.0
    pat4 = pat4.astype(_md.bfloat16)
    gamma_bc = np.tile(np.asarray(gamma, np.float32)[None, :], (128, 1))
    beta_bc = np.tile(np.asarray(beta, np.float32)[None, :], (128, 1))
    b2_bc = np.tile(np.asarray(b2, np.float32)[None, :], (128, 1))

    common = dict(
        xhl_lo=xhl[:half].copy(), xhl_hi=xhl[half:].copy(),
        iota=iota, ident=ident,
        wcb=wcb, w1l2=np.concatenate([np.asarray(W1l, np.float32)] * 2, axis=0),
        w2lr=w2lr,
        gamma_bc=gamma_bc, beta_bc=beta_bc, b2_bc=b2_bc, pat4=pat4,
    )
    in_maps = []
    for c in range(N_CORES):
        m = dict(common)
        xt = xp[cp * c:cp * (c + 1)].T.copy()          # [64, cp]
        xt1 = np.concatenate([xt, np.ones((1, cp), np.float32)], axis=0)
        m["xt1"] = xt1
        m["gidx"] = plan["gidx_tile"][c]
        m["dstf"] = plan["dstf_tile"][c]
        m["rc"] = plan["rc_tile"][c]
        in_maps.append(m)
    return in_maps


_CACHE = {}


def _get_compiled(edge_index, n_nodes):
    key = (edge_index.tobytes()[:512], edge_index.shape, n_nodes)
    if key not in _CACHE:
        plan = make_plan(edge_index, n_nodes)
        nc = build_program(plan)
        _CACHE[key] = (plan, nc)
    return _CACHE[key]


def run(inputs, trace=False):
    x = np.asarray(inputs["x"], np.float32)
    edge_index = np.asarray(inputs["edge_index"], np.int32)
    n_nodes = x.shape[0]
    plan, nc = _get_compiled(edge_index, n_nodes)
    in_maps = make_inputs(
        plan, x, inputs["W1l"], inputs["W1r"], inputs["b1"], inputs["Wskip"],
        inputs["bskip"], inputs["gamma"], inputs["beta"], inputs["W2l"],
        inputs["W2r"], inputs["b2"], n_nodes)
    res = run_bass_kernel_spmd(
        nc, in_maps, list(range(N_CORES)), trace=trace)
    cp = plan["cp"]
    out = np.empty((n_nodes, OUT), np.float32)
    for c in range(N_CORES):
        lo = cp * c
        hi = min(cp * (c + 1), n_nodes)
        out[lo:hi] = res.results[c]["out"][0:hi - lo]
    return out, res


def kernel(**inputs) -> np.ndarray:
    out, _ = run(inputs)
    return out


# revision 24
# speedup vs baseline: 1.5385x; 1.0017x over previous
"""GeneSAGE (2-layer GraphSAGE + skip + LayerNorm + ELU) on 8 Trainium2 cores.

V2 design. Edge-parallel by destination range: core c owns dst nodes
[cp*c, cp*(c+1)), cp=6272. Edges bucketed host-side by (dst-core, src-half,
dst-window) into 128-edge chunks with an SPMD-common chunk schedule.

Key speed choices (measured on HW):
- Gathered x is stored hi/lo bf16 split ([bf16(x) | bf16(x-bf16(x))] 256B
  rows) so the aggregation matmul runs in bf16 (1 instr, ~130ns) while
  keeping fp32-class accuracy (products exact vs 0/1 one-hot; PSUM fp32).
- One-hot built on DVE via is_equal (bf16 out, fp32 iota/scalar) ~160ns.
- Aggregation matmul operand-swapped: lhsT = gathered tile, rhs = one-hot
  -> PSUM holds aggT (feature-major), killing the dense-phase transposes.
- Edge counts (degrees) precomputed on host; no count matmuls.
- dma_gather calls round-robin over 2 SWDGE queues: descriptor generation
  of the next call overlaps the wait of the current (7.9 -> ~4.7 ns/idx).
- conv2 gathers a [p_hi | p_lo]-replicated 256B-row table (built on device
  from an AllGather of p^T hi/lo parts); one bf16 matmul per chunk.
"""

import numpy as np

import concourse.mybir as mybir
from concourse import bacc, bass, tile
from concourse.bass_utils import run_bass_kernel_spmd

F32 = mybir.dt.float32
BF16 = mybir.dt.bfloat16
I16 = mybir.dt.int16

N_CORES = 8
D = 64          # input feature dim
HID = 256
OUT = 2
LN_EPS = 1e-5
BATCH_CHUNKS = 32   # chunks per dma_gather call
NSWQ = 2            # SWDGE queues (2 parallel descriptor-gen cores)


def make_plan(edge_index: np.ndarray, n_nodes: int):
    """Host-side edge bucketing + degree counts. SPMD-common schedule."""
    cp = int(np.ceil(n_nodes / (N_CORES * 128))) * 128
    nw = cp // 128
    npad = N_CORES * cp
    half = npad // 2
    assert half <= 32768, "int16 gather index limit"

    src = edge_index[0].astype(np.int64)
    dst = edge_index[1].astype(np.int64)
    E = src.shape[0]

    core = dst // cp
    stream = (src >= half).astype(np.int64)
    win = (dst % cp) // 128
    ngrp_per_core = 2 * nw
    key = (core * 2 + stream) * nw + win
    order = np.argsort(key, kind="stable")
    counts = np.bincount(key, minlength=N_CORES * ngrp_per_core).reshape(
        N_CORES, 2, nw
    )
    nchunks = -(-counts.max(axis=0) // 128)  # [2, nw]
    off = np.zeros((2, nw), np.int64)
    running = 0
    for s in range(2):
        for w in range(nw):
            off[s, w] = running
            running += nchunks[s, w]
    c_total = int(running)
    e_slots = c_total * 128

    sk = key[order]
    grp_start = np.searchsorted(sk, np.arange(N_CORES * ngrp_per_core))
    rank = np.arange(E) - grp_start[sk]
    s_of = (sk // nw) % 2
    w_of = sk % nw
    c_of = sk // ngrp_per_core
    slot = off[s_of, w_of] * 128 + rank

    gidx = np.zeros((N_CORES, e_slots), np.int16)
    dstf = np.full((N_CORES, e_slots), -1.0, np.float32)
    gidx[c_of, slot] = (src[order] - s_of * half).astype(np.int16)
    dstf[c_of, slot] = (dst[order] % cp - w_of * 128).astype(np.float32)

    # gather idx tile [128, e_slots//16*8]: tile[p, j] = gidx[16*j + p%16]
    a = gidx.reshape(N_CORES, e_slots // 16, 16).transpose(0, 2, 1)
    gidx_tile = np.tile(a, (1, 8, 1)).copy()  # [c, 128, J]
    dstf_tile = (
        dstf.reshape(N_CORES, c_total, 128).transpose(0, 2, 1).copy()
    )  # [c, 128, C]

    sched = []
    for s in range(2):
        rows = []
        for w in range(nw):
            n = int(nchunks[s, w])
            if n == 0:
                continue
            first = int(off[s, w])
            rows.append((w, first, first + n - 1))
        sched.append(rows)

    # degree counts per node (for mean); reciprocal, per-core window tiles
    cnt = np.bincount(dst, minlength=npad).astype(np.float32)
    rc = 1.0 / np.maximum(cnt, 1.0)
    # rc_tile[c][p, w] = rc[cp*c + 128*w + p]
    rc_tile = rc.reshape(N_CORES, nw, 128).transpose(0, 2, 1).copy()
    # rc broadcast tile for aggT scaling: [128, cp] where [p, 128*w+j] =
    # rc[cp*c + 128*w + j]  (same value down all partitions)
    rc_bc = np.broadcast_to(
        rc.reshape(N_CORES, 1, cp), (N_CORES, 128, cp)
    ).copy()

    return dict(
        cp=cp, nw=nw, npad=npad, half=half,
        c_total=c_total,
        sched=sched,
        gidx_tile=gidx_tile, dstf_tile=dstf_tile,
        rc_tile=rc_tile, rc_bc=rc_bc,
    )


def build_program(plan):
    cp, nw, half = plan["cp"], plan["nw"], plan["half"]
    c_total = plan["c_total"]
    sched = plan["sched"]
    J = c_total * 8
    half_w = half // 128  # 128-row blocks per half table

    nc = bacc.Bacc("TRN2", target_bir_lowering=False, debug=False,
                   num_devices=N_CORES, num_swdge_queues=NSWQ)

    def inp(name, shape, dt=F32):
        return nc.dram_tensor(name, shape, dt, kind="ExternalInput").ap()

    xhl_lo = inp("xhl_lo", [half, 128], BF16)   # [x_hi | x_lo] rows
    xhl_hi = inp("xhl_hi", [half, 128], BF16)
    gidx_d = inp("gidx", [128, J], I16)
    dstf_d = inp("dstf", [128, c_total])
    iota_d = inp("iota", [128, 128])
    xt1_d = inp("xt1", [D + 1, cp])             # x_loc^T with ones row
    wcb_d = inp("wcb", [D + 1, HID])            # [Wc; bc] fp32
    w1l2_d = inp("w1l2", [2 * D, HID])
    w2lr_d = inp("w2lr", [128, 2 * 2 * OUT])    # packed halves [W2l|W2r]
    ident_d = inp("ident", [128, 128])
    gamma_d = inp("gamma_bc", [128, HID])
    beta_d = inp("beta_bc", [128, HID])
    b2_d = inp("b2_bc", [128, OUT])
    rc_d = inp("rc", [128, nw])
    rcbc_d = inp("rc_bc", [128, cp])
    pat4_d = inp("pat4", [4, 128], BF16)              # p4 tiling pattern
    out_d = nc.dram_tensor("out", [cp, OUT], F32, kind="ExternalOutput").ap()

    with tile.TileContext(nc) as tc:
        with (
            tc.tile_pool(name="res", bufs=1) as res,
            tc.tile_pool(name="dram", bufs=1, space="DRAM") as dram,
        ):
            # ---- resident tiles
            gidx_sb = res.tile([128, J], I16)
            nc.sync.dma_start(out=gidx_sb[:], in_=gidx_d[:])
            dstf_sb = res.tile([128, c_total], F32)
            nc.sync.dma_start(out=dstf_sb[:], in_=dstf_d[:])
            iota_sb = res.tile([128, 128], F32)
            nc.sync.dma_start(out=iota_sb[:], in_=iota_d[:])
            ident_sb = res.tile([128, 128], F32)
            nc.sync.dma_start(out=ident_sb[:], in_=ident_d[:])
            xt1_sb = res.tile([D + 1, nw, 128], F32)
            nc.sync.dma_start(
                out=xt1_sb[:], in_=xt1_d.rearrange("f (w p) -> f w p", p=128))
            wcb_sb = res.tile([D + 1, HID], F32)
            nc.sync.dma_start(out=wcb_sb[:], in_=wcb_d[:])
            w1l2_sb = res.tile([2 * D, HID], F32)
            nc.sync.dma_start(out=w1l2_sb[:], in_=w1l2_d[:])
            w2lr_sb = res.tile([128, 2 * 2 * OUT], F32)
            nc.sync.dma_start(out=w2lr_sb[:], in_=w2lr_d[:])
            gamma_sb = res.tile([128, HID], F32)
            nc.sync.dma_start(out=gamma_sb[:], in_=gamma_d[:])
            beta_sb = res.tile([128, HID], F32)
            nc.sync.dma_start(out=beta_sb[:], in_=beta_d[:])
            b2_sb = res.tile([128, OUT], F32)
            nc.sync.dma_start(out=b2_sb[:], in_=b2_d[:])
            rc_sb = res.tile([128, nw], F32)
            nc.sync.dma_start(out=rc_sb[:], in_=rc_d[:])
            rcbc_sb = res.tile([128, nw, 128], F32)
            nc.sync.dma_start(
                out=rcbc_sb[:],
                in_=rcbc_d.rearrange("p (w j) -> p w j", j=128))
            pat4_sb = res.tile([4, 128], BF16)
            nc.sync.dma_start(out=pat4_sb[:], in_=pat4_d[:])

            pr_sb = res.tile([128, nw, 2 * OUT], F32)
            out_sb = res.tile([128, nw, OUT], F32)
            pt4_sb = res.tile([4, cp], BF16)

            pt4_dram = dram.tile([4, cp], BF16)
            pt4all_dram = dram.tile([4 * N_CORES, cp], BF16)
            pb2_lo = dram.tile([half, 128], BF16)
            pb2_hi = dram.tile([half, 128], BF16)

            # ---- aggregation pass: edge stream -> per-window PSUM aggT
            def aggregation(tables, wout, sink):
                """tables: (lo, hi) DRAM [half, 128] bf16 with rows
                [v_hi | v_lo | ...] (wout cols used per chunk matmul).
                For each window w calls sink(s, w, pw) with pw [wout, 128]
                PSUM = transposed aggregate over the window's chunks (one
                bf16 matmul per chunk: lhsT = gathered cols 0:wout, rhs =
                dst one-hot). hi/lo parts are NOT summed here; the
                consumer's contraction or a later transpose handles it.
                iota lives in PSUM so the is_eq runs in 1-port mode and
                does not contend with SWDGE descriptor traffic.
                """
                with (
                    tc.tile_pool(name="gpool", bufs=3) as gpool,
                    tc.tile_pool(name="opool", bufs=2) as opool,
                    tc.tile_pool(name="pwpool", bufs=2, space="PSUM") as pwp,
                    tc.tile_pool(name="ipool", bufs=1, space="PSUM") as ipl,
                ):
                    iota_psum = ipl.tile([128, 128], F32, tag="iop")
                    nc.scalar.activation(
                        iota_psum[:], iota_sb[:],
                        mybir.ActivationFunctionType.Copy)
                    qn = 0
                    for s in range(2):
                        table = tables[s]
                        rows = sched[s]
                        if not rows:
                            continue
                        c0 = rows[0][1]
                        c1 = rows[-1][2] + 1
                        gbufs = {}
                        for b0 in range(c0, c1, BATCH_CHUNKS):
                            b1 = min(b0 + BATCH_CHUNKS, c1)
                            g = gpool.tile([128, BATCH_CHUNKS, 128], BF16,
                                           tag="gbuf")
                            n_idx = (b1 - b0) * 128
                            nc.gpsimd.dma_gather(
                                out_ap=g[:, 0:b1 - b0, :],
                                in_ap=table,
                                idxs_ap=gidx_sb[:, b0 * 8:b1 * 8],
                                num_idxs=n_idx,
                                num_idxs_reg=n_idx,
                                elem_size=128,
                                single_packet=False,
                                queue_num=qn)
                            qn = (qn + 1) % NSWQ
                            gbufs[b0] = g
                        for w, first, last in rows:
                            pw = pwp.tile([wout, 128], F32, tag="pw")
                            for gci in range(first, last + 1):
                                b0 = c0 + ((gci - c0) // BATCH_CHUNKS) \
                                    * BATCH_CHUNKS
                                gb = gbufs[b0]
                                o = opool.tile([128, 128], BF16, tag="O")
                                nc.vector.tensor_scalar(
                                    out=o[:],
                                    in0=iota_psum[:],
                                    scalar1=dstf_sb[:, gci:gci + 1],
                                    scalar2=None,
                                    op0=mybir.AluOpType.is_equal)
                                nc.tensor.matmul(
                                    pw[:], gb[:, gci - b0, 0:wout], o[:],
                                    start=(gci == first), stop=(gci == last))
                            sink(s, w, pw)

            # conv1 aggT accumulator in SBUF: [128, nw, 128] f32:
            # rows 0:64 hi-part, 64:128 lo-part (summed over streams);
            # the dense matmul against [W1l; W1l] sums hi+lo via its
            # contraction.
            aggT1 = res.tile([128, nw, 128], F32)
            nc.vector.memset(aggT1[:], 0.0)

            # per-window last stream holding chunks (to fire the dense /
            # output phase as soon as that window's aggregate completes,
            # overlapping it with the remaining gather stream)
            last_s = {}
            for s_ in range(2):
                for (w_, _f, _l) in sched[s_]:
                    last_s[w_] = s_

            # ================= conv1 + interleaved dense =================
            with (
                tc.tile_pool(name="dwork", bufs=2) as dwork,
                tc.tile_pool(name="dsmall", bufs=2) as dsmall,
                tc.tile_pool(name="dpsum", bufs=1, space="PSUM") as dpsum,
                tc.tile_pool(name="dpsum2", bufs=2, space="PSUM") as dpsum2,
                tc.tile_pool(name="dpsum3", bufs=1, space="PSUM") as dpsum3,
            ):
                def dense_window(n):
                    # x1 = x@Wc + bc + ((aggT_hi+aggT_lo)*rc)@W1l
                    # ([W1l; W1l] contraction sums hi+lo; rc applied on
                    # the aggregate columns first)
                    am = dwork.tile([128, 128], F32, tag="am")
                    nc.vector.tensor_tensor(
                        out=am[:], in0=aggT1[:, n, :],
                        in1=rcbc_sb[:, n, :], op=mybir.AluOpType.mult)
                    x1p = dpsum2.tile([128, HID], F32, tag="x1")
                    nc.tensor.matmul(x1p[:], xt1_sb[:, n, :], wcb_sb[:],
                                     start=True, stop=False)
                    nc.tensor.matmul(x1p[:], am[:], w1l2_sb[:],
                                     start=False, stop=True)

                    # LayerNorm + ELU
                    mu = dsmall.tile([128, 1], F32, tag="mu")
                    nc.vector.reduce_sum(out=mu[:], in_=x1p[:],
                                         axis=mybir.AxisListType.X)
                    nc.vector.tensor_scalar(
                        out=mu[:], in0=mu[:], scalar1=1.0 / HID,
                        scalar2=None, op0=mybir.AluOpType.mult)
                    xc = dwork.tile([128, HID], F32, tag="xc")
                    nc.vector.tensor_scalar(
                        out=xc[:], in0=x1p[:], scalar1=mu[:], scalar2=None,
                        op0=mybir.AluOpType.subtract)
                    sq = dwork.tile([128, HID], F32, tag="sq")
                    var = dsmall.tile([128, 1], F32, tag="var")
                    nc.scalar.activation(
                        sq[:], xc[:], mybir.ActivationFunctionType.Square,
                        accum_out=var[:])
                    rstd = dsmall.tile([128, 1], F32, tag="rstd")
                    nc.vector.tensor_scalar(
                        out=rstd[:], in0=var[:], scalar1=1.0 / HID,
                        scalar2=LN_EPS, op0=mybir.AluOpType.mult,
                        op1=mybir.AluOpType.add)
                    nc.scalar.activation(
                        rstd[:], rstd[:], mybir.ActivationFunctionType.Sqrt)
                    nc.vector.reciprocal(rstd[:], rstd[:])
                    y = dwork.tile([128, HID], F32, tag="y")
                    nc.scalar.activation(
                        y[:], xc[:], mybir.ActivationFunctionType.Copy,
                        scale=rstd[:])
                    nc.vector.tensor_tensor(
                        out=y[:], in0=y[:], in1=gamma_sb[:],
                        op=mybir.AluOpType.mult)
                    nc.vector.tensor_tensor(
                        out=y[:], in0=y[:], in1=beta_sb[:],
                        op=mybir.AluOpType.add)
                    # ELU: h = max(y,0)-1 + exp(min(y,0))
                    m0 = dwork.tile([128, HID], F32, tag="m0")
                    nc.vector.tensor_scalar(
                        out=m0[:], in0=y[:], scalar1=0.0, scalar2=None,
                        op0=mybir.AluOpType.min)
                    ex = dwork.tile([128, HID], F32, tag="ex")
                    nc.scalar.activation(
                        ex[:], m0[:], mybir.ActivationFunctionType.Exp)
                    rm1 = dwork.tile([128, HID], F32, tag="rm1")
                    nc.vector.tensor_scalar(
                        out=rm1[:], in0=y[:], scalar1=0.0, scalar2=-1.0,
                        op0=mybir.AluOpType.max, op1=mybir.AluOpType.add)
                    hwin = dwork.tile([128, HID], F32, tag="hwin")
                    nc.vector.tensor_tensor(
                        out=hwin[:], in0=rm1[:], in1=ex[:],
                        op=mybir.AluOpType.add)

                    # pr = h @ [W2l | W2r]  [128, 4] fp32
                    prp = dpsum3.tile([128, 2 * OUT], F32, tag="pr")
                    for hh in range(2):
                        tph = dpsum.tile([128, 128], F32, tag="tph")
                        nc.tensor.transpose(
                            tph[:], hwin[:, 128 * hh:128 * (hh + 1)],
                            ident_sb[:])
                        hts = dwork.tile([128, 128], F32, tag="hts")
                        nc.scalar.activation(
                            hts[:], tph[:],
                            mybir.ActivationFunctionType.Copy)
                        nc.tensor.matmul(
                            prp[:], hts[:],
                            w2lr_sb[:, 4 * hh:4 * (hh + 1)],
                            start=(hh == 0), stop=(hh == 1))
                    nc.scalar.activation(
                        pr_sb[:, n, :], prp[:],
                        mybir.ActivationFunctionType.Copy)

                    # p4 = [p_hi (2) | p_lo (2)] from p = pr[:, 0:2]
                    p_hi_b = dsmall.tile([128, OUT], BF16, tag="phb")
                    nc.vector.tensor_copy(p_hi_b[:], pr_sb[:, n, 0:OUT])
                    p_hi_f = dsmall.tile([128, OUT], F32, tag="phf")
                    nc.vector.tensor_copy(p_hi_f[:], p_hi_b[:])
                    p4 = dwork.tile([128, 2 * OUT], F32, tag="p4")
                    nc.vector.tensor_copy(p4[:, 0:OUT], p_hi_f[:])
                    nc.vector.tensor_tensor(
                        out=p4[:, OUT:2 * OUT], in0=pr_sb[:, n, 0:OUT],
                        in1=p_hi_f[:], op=mybir.AluOpType.subtract)
                    # pt4[:, w*128:...] = p4^T
                    ptp = dpsum.tile([2 * OUT, 128], F32, tag="ptp")
                    nc.tensor.transpose(ptp[:], p4[:], ident_sb[:])
                    nc.scalar.activation(
                        pt4_sb[:, 128 * n:128 * (n + 1)], ptp[:],
                        mybir.ActivationFunctionType.Copy)

                def sink1(s, w, pw):
                    nc.vector.tensor_tensor(
                        out=aggT1[:, w, :], in0=aggT1[:, w, :], in1=pw[:],
                        op=mybir.AluOpType.add)
                    if last_s.get(w) == s:
                        dense_window(w)

                aggregation((xhl_lo, xhl_hi), 128, sink1)
                for n in range(nw):
                    if n not in last_s:
                        dense_window(n)

                nc.sync.dma_start(out=pt4_dram[:], in_=pt4_sb[:])

            # ================= p4 all-gather =================
            nc.gpsimd.collective_compute(
                "AllGather",
                mybir.AluOpType.bypass,
                replica_groups=[list(range(N_CORES))],
                ins=[pt4_dram.opt()],
                outs=[pt4all_dram.opt()],
            )

            # ============ build pb2 (replicated p4 table, bf16) ============
            n_glob = N_CORES * nw
            with (
                tc.tile_pool(name="bstage", bufs=3) as bstage,
                tc.tile_pool(name="bpt", bufs=1) as bpt,
                tc.tile_pool(name="bpsum", bufs=2, space="PSUM") as bpsum,
            ):
                stage_n = 7  # 49 % 7 == 0; half_w = 196 = 28*7
                stage = None
                ptb = None
                for j in range(n_glob):
                    c = j // nw
                    jw = j % nw
                    if jw == 0:
                        ptb = bpt.tile([4, cp], BF16, tag="ptb", name="ptb")
                        nc.sync.dma_start(
                            out=ptb[:], in_=pt4all_dram[4 * c:4 * c + 4, :])
                    pp = bpsum.tile([128, 128], F32, tag="pb2p")
                    nc.tensor.matmul(
                        pp[:],
                        ptb[:, 128 * jw:128 * (jw + 1)],
                        pat4_sb[:], start=True, stop=True)
                    if j % stage_n == 0:
                        stage = bstage.tile([128, stage_n, 128], BF16,
                                            tag="stage")
                    if j % 2 == 0:
                        nc.scalar.activation(
                            stage[:, j % stage_n, :], pp[:],
                            mybir.ActivationFunctionType.Copy)
                    else:
                        nc.vector.tensor_copy(
                            stage[:, j % stage_n, :], pp[:])
                    if j % stage_n == stage_n - 1:
                        j0 = j - stage_n + 1
                        r0 = j0 * 128
                        if r0 < half:
                            dst = pb2_lo[r0:r0 + stage_n * 128, :]
                        else:
                            dst = pb2_hi[r0 - half:r0 - half
                                         + stage_n * 128, :]
                        nc.sync.dma_start(
                            out=dst.rearrange("(s p) d -> p s d", p=128),
                            in_=stage[:])

            # ============ conv2 aggregation + interleaved output ============
            # out[dst, c] = (agg2_hi + agg2_lo)[c, dst] * rc + r + b2
            agg2T = res.tile([2 * OUT, nw, 128], F32)
            nc.vector.memset(agg2T[:], 0.0)

            with (
                tc.tile_pool(name="fsmall", bufs=2) as fsmall,
                tc.tile_pool(name="fpsum", bufs=2, space="PSUM") as fpsum,
            ):
                def out_window(n):
                    # transpose [4, 128] -> [128, 4]
                    a2t = fpsum.tile([128, 2 * OUT], F32, tag="a2t")
                    nc.tensor.transpose(a2t[:], agg2T[:, n, :],
                                        ident_sb[0:2 * OUT, 0:2 * OUT])
                    a4s = fsmall.tile([128, 2 * OUT], F32, tag="a4s")
                    nc.scalar.activation(
                        a4s[:], a2t[:], mybir.ActivationFunctionType.Copy)
                    asum = fsmall.tile([128, OUT], F32, tag="as")
                    nc.vector.tensor_tensor(
                        out=asum[:], in0=a4s[:, 0:OUT],
                        in1=a4s[:, OUT:2 * OUT], op=mybir.AluOpType.add)
                    t = fsmall.tile([128, OUT], F32, tag="fo")
                    nc.vector.tensor_scalar(
                        out=t[:], in0=asum[:], scalar1=rc_sb[:, n:n + 1],
                        scalar2=None, op0=mybir.AluOpType.mult)
                    nc.vector.tensor_tensor(
                        out=t[:], in0=t[:], in1=pr_sb[:, n, OUT:2 * OUT],
                        op=mybir.AluOpType.add)
                    nc.vector.tensor_tensor(
                        out=out_sb[:, n, :], in0=t[:], in1=b2_sb[:],
                        op=mybir.AluOpType.add)

                def sink2(s, w, pw):
                    nc.vector.tensor_tensor(
                        out=agg2T[:, w, :], in0=agg2T[:, w, :],
                        in1=pw[:], op=mybir.AluOpType.add)
                    if last_s.get(w) == s:
                        out_window(w)

                aggregation((pb2_lo, pb2_hi), 2 * OUT, sink2)
                for n in range(nw):
                    if n not in last_s:
                        out_window(n)
            nc.sync.dma_start(
                out=out_d.rearrange("(w p) c -> p w c", p=128),
                in_=out_sb[:])

    nc.compile()
    return nc


def make_inputs(plan, x, W1l, W1r, b1, Wskip, bskip, gamma, beta, W2l, W2r,
                b2, n_nodes):
    import ml_dtypes
    cp, half, npad, nw = plan["cp"], plan["half"], plan["npad"], plan["nw"]
    xp = np.zeros((npad, D), np.float32)
    xp[:n_nodes] = np.asarray(x, np.float32)
    x_hi = xp.astype(ml_dtypes.bfloat16)
    x_lo = (xp - x_hi.astype(np.float32)).astype(ml_dtypes.bfloat16)
    xhl = np.concatenate([x_hi, x_lo], axis=1)  # [npad, 128] bf16

    wc = np.asarray(W1r, np.float32) + np.asarray(Wskip, np.float32)
    bc = np.asarray(b1, np.float32) + np.asarray(bskip, np.float32)
    wcb = np.concatenate([wc, bc[None, :]], axis=0)
    w2lr_full = np.concatenate(
        [np.asarray(W2l, np.float32), np.asarray(W2r, np.float32)], axis=1
    )  # [HID, 4]
    w2lr = (
        w2lr_full.reshape(2, 128, 2 * OUT).transpose(1, 0, 2)
        .reshape(128, 2 * 2 * OUT).copy()
    )
    iota = np.tile(np.arange(128, dtype=np.float32)[None, :], (128, 1))
    ident = np.eye(128, dtype=np.float32)
    # pat4[q, 128 cols] = 1 where col % 4 == q
    import ml_dtypes as _md
    pat4 = np.zeros((4, 128), np.float32)
    for q in range(4):
        pat4[q, q::4] = 1.0
    pat4 = pat4.astype(_md.bfloat16)
    gamma_bc = np.tile(np.asarray(gamma, np.float32)[None, :], (128, 1))
    beta_bc = np.tile(np.asarray(beta, np.float32)[None, :], (128, 1))
    b2_bc = np.tile(np.asarray(b2, np.float32)[None, :], (128, 1))

    common = dict(
        xhl_lo=xhl[:half].copy(), xhl_hi=xhl[half:].copy(),
        iota=iota, ident=ident,
        wcb=wcb, w1l2=np.concatenate([np.asarray(W1l, np.float32)] * 2, axis=0),
        w2lr=w2lr,
        gamma_bc=gamma_bc, beta_bc=beta_bc, b2_bc=b2_bc, pat4=pat4,
    )
    in_maps = []
    for c in range(N_CORES):
        m = dict(common)
        xt = xp[cp * c:cp * (c + 1)].T.copy()          # [64, cp]
        xt1 = np.concatenate([xt, np.ones((1, cp), np.float32)], axis=0)
        m["xt1"] = xt1
        m["gidx"] = plan["gidx_tile"][c]
        m["dstf"] = plan["dstf_tile"][c]
        m["rc"] = plan["rc_tile"][c]
        m["rc_bc"] = plan["rc_bc"][c]
        in_maps.append(m)
    return in_maps


_CACHE = {}


def _get_compiled(edge_index, n_nodes):
    key = (edge_index.tobytes()[:512], edge_index.shape, n_nodes)
    if key not in _CACHE:
        plan = make_plan(edge_index, n_nodes)
        nc = build_program(plan)
        _CACHE[key] = (plan, nc)
    return _CACHE[key]


def run(inputs, trace=False):
    x = np.asarray(inputs["x"], np.float32)
    edge_index = np.asarray(inputs["edge_index"], np.int32)
    n_nodes = x.shape[0]
    plan, nc = _get_compiled(edge_index, n_nodes)
    in_maps = make_inputs(
        plan, x, inputs["W1l"], inputs["W1r"], inputs["b1"], inputs["Wskip"],
        inputs["bskip"], inputs["gamma"], inputs["beta"], inputs["W2l"],
        inputs["W2r"], inputs["b2"], n_nodes)
    res = run_bass_kernel_spmd(
        nc, in_maps, list(range(N_CORES)), trace=trace)
    cp = plan["cp"]
    out = np.empty((n_nodes, OUT), np.float32)
    for c in range(N_CORES):
        lo = cp * c
        hi = min(cp * (c + 1), n_nodes)
        out[lo:hi] = res.results[c]["out"][0:hi - lo]
    return out, res


def kernel(**inputs) -> np.ndarray:
    out, _ = run(inputs)
    return out
